# revision 9
# baseline (speedup 1.0000x reference)
"""Trainium2 Bass kernel for ChaoticAttentionLayer.

Math (reference):
    q = r_s * sig(zq) * (1 - sig(zq)),  zq = query @ Wq.T + bq,  r_s = 4*sigmoid(r)
    k likewise, v = value @ Wv.T + bv
    out = softmax(q k^T / 8) v @ Wo.T + bo   (per head, D=64)

Device decomposition:
    g = sig*(1-sig); scores = (r_s^2/8) * g(zq) . g(zk); the r_s^2/8 factor is
    folded into the Exp activation's scale. Scores are bounded in [0, 8] for
    any r, so softmax runs max-free: exp(scores) directly, denominator via an
    all-ones column appended to V.

Sharding: 8 cores = 4 batches x 2 head-groups (4 heads each). Each core
computes partial out[b] = attn_hg @ Wo[:, hg].T; host sums the two partials
per batch and adds bo.

v3 structure (vs the original baseline):
  - All projections in a prologue; K/Q projections accumulate a full
    [128, 2048] psum tile (4 banks) so the Tanh activation runs at N=2048
    (352-cycle ACT fixed cost amortized 4x better).
  - Attention emission is software-pipelined: the AV matmuls for score
    block sb are emitted AFTER the QK matmuls of block sb+1, so the PE
    never head-of-line blocks on the Exp of the current block. This keeps
    the PE stream dense, which keeps the HAM clock-gate at K=8/8 (the
    baseline oscillated and ran the PE at 1.2 GHz for ~57% of the kernel).
  - Exp stays at N=1024 per call (PSUM bank budget forbids larger), which
    makes ScalarE the pacing engine at ~147us.
"""

import numpy as np
import ml_dtypes
from contextlib import ExitStack

try:
    import concourse.bass as bass
except ImportError:  # pragma: no cover
    import sys

    sys.path.insert(0, "/opt/trn_rl_repo")
    import concourse.bass as bass

import concourse.bacc as bacc
import concourse.tile as tile
from concourse import mybir
from concourse.bass_utils import run_bass_kernel_spmd
from concourse.masks import make_identity

F32 = mybir.dt.float32
BF16 = mybir.dt.bfloat16
AF = mybir.ActivationFunctionType
BF16NP = ml_dtypes.bfloat16

B, T, S, E, H = 4, 2048, 2048, 512, 8
D = E // H           # 64 head dim
HG = 2               # head-groups per batch (cores per batch)
HPG = H // HG        # 4 heads per group
EG = HPG * D         # 256 dims per head group
NCORES = 8
P = 128              # partitions
TCH = 512            # t-chunk (psum free dim)
NSB = S // P         # 16 s-blocks
NKT = E // P         # 4 contraction tiles of 128
NTC = T // TCH       # 4 t-chunks


def _build():
    nc = bacc.Bacc("TRN2", target_bir_lowering=False, debug=False,
                   num_devices=NCORES)

    xqT = nc.dram_tensor("xqT", [E, T], BF16, kind="ExternalInput")
    xkT = nc.dram_tensor("xkT", [E, S], BF16, kind="ExternalInput")
    xvT = nc.dram_tensor("xvT", [E + 1, S], BF16, kind="ExternalInput")
    wqT = nc.dram_tensor("wqT", [E, EG], BF16, kind="ExternalInput")
    wkT = nc.dram_tensor("wkT", [E, EG], BF16, kind="ExternalInput")
    wvT = nc.dram_tensor("wvT", [E + 1, EG], BF16, kind="ExternalInput")
    woT = nc.dram_tensor("woT", [EG, E], BF16, kind="ExternalInput")
    bq = nc.dram_tensor("bq", [EG, 1], F32, kind="ExternalInput")
    bk = nc.dram_tensor("bk", [EG, 1], F32, kind="ExternalInput")
    cexp = nc.dram_tensor("cexp", [1, 1], F32, kind="ExternalInput")
    out = nc.dram_tensor("out", [T, E], F32, kind="ExternalOutput")

    with tile.TileContext(nc) as tc, ExitStack() as ctx:
        persist = ctx.enter_context(tc.tile_pool(name="persist", bufs=1))

        # DMA issue engines round-robin: each engine's dma_start lands on its
        # own hardware queue, so input loads run on 3 queues in parallel
        # (a single queue sustains only ~160 GB/s).
        dma_engs = [nc.sync, nc.gpsimd, nc.scalar]

        # --- persistent SBUF state ---
        # K-projection inputs first: they gate the whole pipeline.
        # x inputs are full-row [128, 2048] tiles (4 KB contiguous rows).
        wk_sb = []
        bk_sb = []
        for kt in range(NKT):
            tk = persist.tile([P, EG], BF16, tag=f"wk{kt}")
            dma_engs[kt % 3].dma_start(out=tk, in_=wkT[kt * P:(kt + 1) * P, :])
            wk_sb.append(tk)
        for c in range(EG // P):
            tb2 = persist.tile([P, 1], F32, tag=f"bk{c}")
            nc.sync.dma_start(out=tb2, in_=bk[c * P:(c + 1) * P, :])
            bk_sb.append(tb2)

        xk_sb = []
        xv_sb = []
        xq_sb = []
        for kt in range(NKT):
            xk_t = persist.tile([P, S], BF16, tag=f"xk{kt}", name=f"xk{kt}")
            dma_engs[kt % 3].dma_start(out=xk_t,
                                       in_=xkT[kt * P:(kt + 1) * P, :])
            xk_sb.append(xk_t)

        wq_sb = []
        bq_sb = []
        for kt in range(NKT):
            tq = persist.tile([P, EG], BF16, tag=f"wq{kt}")
            dma_engs[kt % 3].dma_start(out=tq, in_=wqT[kt * P:(kt + 1) * P, :])
            wq_sb.append(tq)
        for c in range(EG // P):
            tb_ = persist.tile([P, 1], F32, tag=f"bq{c}")
            nc.sync.dma_start(out=tb_, in_=bq[c * P:(c + 1) * P, :])
            bq_sb.append(tb_)
        for kt in range(NKT):
            xq_t = persist.tile([P, T], BF16, tag=f"xq{kt}", name=f"xq{kt}")
            dma_engs[kt % 3].dma_start(out=xq_t,
                                       in_=xqT[kt * P:(kt + 1) * P, :])
            xq_sb.append(xq_t)

        wv_sb = []
        for kt in range(NKT):
            tv = persist.tile([P, EG], BF16, tag=f"wv{kt}")
            dma_engs[kt % 3].dma_start(out=tv, in_=wvT[kt * P:(kt + 1) * P, :])
            wv_sb.append(tv)
        wv4_sb = persist.tile([1, EG], BF16, tag="wv4")
        nc.sync.dma_start(out=wv4_sb, in_=wvT[E:E + 1, :])
        for kt in range(NKT):
            xv_t = persist.tile([P, S], BF16, tag=f"xvr{kt}", name=f"xvr{kt}")
            dma_engs[kt % 3].dma_start(out=xv_t,
                                       in_=xvT[kt * P:(kt + 1) * P, :])
            xv_sb.append(xv_t)

        ones_sb = persist.tile([1, S], BF16, tag="ones")
        nc.sync.dma_start(out=ones_sb, in_=xvT[E:E + 1, :])

        cexp_sb = persist.tile([P, 1], F32, tag="cexp")
        cap = cexp[:, :]
        nc.sync.dma_start(
            out=cexp_sb,
            in_=bass.AP(tensor=cap.tensor, offset=cap.offset, ap=[[0, P], [1, 1]]),
        )

        # out-proj weights are only needed ~100us in; identity likewise.
        wo_sb = []
        for kb in range(EG // P):
            to = persist.tile([P, E], BF16, tag=f"wo{kb}")
            nc.sync.dma_start(out=to, in_=woT[kb * P:(kb + 1) * P, :])
            wo_sb.append(to)

        ident = persist.tile([P, P], BF16, tag="ident")
        make_identity(nc, ident)

        # PE warm-up: the HAM clock gate starts at K=4/8 (1.2 GHz) and only
        # reaches 2.4 GHz after ~3.4us of sustained PE activity. While the
        # input DMAs stream in, run dummy matmuls on the identity tile so the
        # projections start at full clock instead of paying the cold tax.
        NWARM = 96

        # projected tensors, resident for the whole kernel.
        # Full-width [P, 2048] tiles: free dim is s (for K) / t (for Q).
        QT_sb = [persist.tile([P, T], BF16, tag=f"qt{c}", name=f"qt{c}")
                 for c in range(EG // P)]
        KT_sb = [persist.tile([P, S], BF16, tag=f"kt{c}", name=f"ktile{c}")
                 for c in range(EG // P)]
        V_sb = [persist.tile([P, HPG, D + 1], BF16, tag=f"v{sc}", name=f"v{sc}")
                for sc in range(NSB)]

        # --- K/Q projections (prologue) ---
        # Accumulate all 4 t-chunks into one [128, 2048] psum tile (4 banks)
        # so Tanh runs at N=2048. V-projection is interleaved into the first
        # attention t-chunk (it borrows the psf bank, which the out-proj does
        # not need until tci=1).
        def kq_proj(x_sb, w_sb, b_sb, out_tiles, pool, sig):
            for c in range(EG // P):
                ps = pool.tile([P, T], F32, tag="pskq", name=f"pskq_{c}")
                for tcq in range(NTC):
                    for kt in range(NKT):
                        nc.tensor.matmul(
                            ps[:, tcq * TCH:(tcq + 1) * TCH],
                            w_sb[kt][:, c * P:(c + 1) * P],
                            x_sb[kt][:, tcq * TCH:(tcq + 1) * TCH],
                            start=(kt == 0), stop=(kt == NKT - 1))
                # sig'(z) = (1 - tanh^2(z/2)) / 4 -- tanh shares ACT's exp
                # table set, so the whole kernel needs one ACT_TABLE_LOAD.
                y = sig.tile([P, T], F32, tag="y", name=f"y_{c}")
                nc.scalar.activation(y, ps, AF.Tanh, bias=b_sb[c], scale=0.5)
                y2 = sig.tile([P, T], F32, tag="y2", name=f"y2_{c}")
                nc.vector.tensor_mul(y2, y, y)
                nc.vector.tensor_scalar(out_tiles[c], y2, -0.25, 0.25,
                                        mybir.AluOpType.mult,
                                        mybir.AluOpType.add)

        with ExitStack() as cp:
            pkq = cp.enter_context(
                tc.tile_pool(name="pkq", bufs=2, space="PSUM"))
            sig = cp.enter_context(tc.tile_pool(name="sig", bufs=2))
            warm = pkq.tile([P, T], F32, tag="pskq", name="warm")
            for _ in range(NWARM):
                nc.tensor.matmul(warm[:, 0:P], ident, ident,
                                 start=True, stop=True)
            kq_proj(xk_sb, wk_sb, bk_sb, KT_sb, pkq, sig)
            kq_proj(xq_sb, wq_sb, bq_sb, QT_sb, pkq, sig)

        # --- attention + out-projection ---
        # Per (tci, hp): 16 score blocks. Emission is software-pipelined:
        # AV for block sb-1 is emitted after the QK pair of block sb, so the
        # PE stream never blocks on the Exp of the block just produced.
        with ExitStack() as c4:
            pss = c4.enter_context(
                tc.tile_pool(name="pss", bufs=2, space="PSUM"))
            psav = c4.enter_context(
                tc.tile_pool(name="psav", bufs=1, space="PSUM"))
            pst = c4.enter_context(
                tc.tile_pool(name="pst", bufs=1, space="PSUM"))
            psf = c4.enter_context(
                tc.tile_pool(name="psf", bufs=1, space="PSUM"))
            expp = c4.enter_context(tc.tile_pool(name="expp", bufs=6))
            attnp = c4.enter_context(tc.tile_pool(name="attnp", bufs=3))
            atp = c4.enter_context(tc.tile_pool(name="atp", bufs=2))
            rdp = c4.enter_context(tc.tile_pool(name="rdp", bufs=8))
            outp = c4.enter_context(tc.tile_pool(name="outp", bufs=3))

            def emit_qk(tci, hp, sb):
                ps = pss.tile([P, 2 * TCH], F32, tag="sc",
                              name=f"ps_{tci}_{hp}_{sb}")
                for hi in range(2):
                    h = 2 * hp + hi
                    ch, off = h // 2, (h % 2) * D
                    nc.tensor.matmul(
                        ps[:, hi * TCH:(hi + 1) * TCH],
                        KT_sb[ch][off:off + D, sb * P:(sb + 1) * P],
                        QT_sb[ch][off:off + D, tci * TCH:(tci + 1) * TCH],
                        start=True, stop=True,
                        tile_position=(off, 0))
                ex = expp.tile([P, 2 * TCH], BF16, tag="ex")
                nc.scalar.activation(ex, ps, AF.Exp, scale=cexp_sb)
                return ex

            def vproj(sc):
                # V-projection for s-block sc; borrows the psf bank, which
                # out-proj does not touch until tci=1.
                ps = psf.tile([P, E], F32, tag="pf", name=f"psv_{sc}")
                for kt in range(NKT):
                    nc.tensor.matmul(ps[:, 0:EG],
                                     xv_sb[kt][:, sc * P:(sc + 1) * P],
                                     wv_sb[kt], start=(kt == 0), stop=False)
                nc.tensor.matmul(ps[:, 0:EG], ones_sb[:, sc * P:(sc + 1) * P],
                                 wv4_sb, start=False, stop=True)
                nc.vector.tensor_copy(
                    V_sb[sc][:, :, 0:D],
                    ps[:, 0:EG].rearrange("p (h d) -> p h d", h=HPG))
                nc.vector.memset(V_sb[sc][:, :, D:D + 1], 1.0)

            def emit_av(avs, hp, sb, ex):
                for hi in range(2):
                    for ts in range(4):
                        nc.tensor.matmul(
                            avs[hi][:, ts * (D + 1):(ts + 1) * (D + 1)],
                            ex[:, hi * TCH + ts * P:hi * TCH + (ts + 1) * P],
                            V_sb[sb][:, 2 * hp + hi, :],
                            start=(sb == 0 and ts == 0),
                            stop=(sb == NSB - 1),
                            skip_group_check=not (sb == 0 and ts == 0))

            def emit_norm(avs, hp, attn_ts):
                # normalization: per head/t-sub per-partition 1/denom
                for hi in range(2):
                    h = 2 * hp + hi
                    for ts in range(4):
                        col = ts * (D + 1)
                        rd = rdp.tile([P, 1], F32, tag="rd")
                        nc.vector.reciprocal(
                            rd, avs[hi][:, col + D:col + D + 1])
                        nc.vector.tensor_scalar_mul(
                            attn_ts[ts][:, h * D:(h + 1) * D],
                            avs[hi][:, col:col + D], rd)

            def outproj_tci(tci, attn_ts):
                aT = [atp.tile([P, TCH], BF16, tag=f"at{kb}",
                               name=f"at_{tci}_{kb}")
                      for kb in range(EG // P)]
                for ts in range(4):
                    for kb in range(EG // P):
                        pt = pst.tile([P, P], BF16, tag="pt",
                                      name=f"pt_{tci}_{ts}_{kb}")
                        nc.tensor.transpose(
                            pt, attn_ts[ts][:, kb * P:(kb + 1) * P], ident)
                        nc.vector.tensor_copy(aT[kb][:, ts * P:(ts + 1) * P], pt)
                for ts in range(4):
                    pf = psf.tile([P, E], F32, tag="pf", name=f"pf_{tci}_{ts}")
                    for kb in range(EG // P):
                        nc.tensor.matmul(pf, aT[kb][:, ts * P:(ts + 1) * P],
                                         wo_sb[kb], start=(kb == 0),
                                         stop=(kb == EG // P - 1))
                    ot = outp.tile([P, E], F32, tag="ot")
                    nc.vector.tensor_copy(ot, pf)
                    row = (tci * 4 + ts) * P
                    dma_engs[(tci * 4 + ts) % 3].dma_start(
                        out=out[row:row + P, :], in_=ot)

            # Flat software pipeline over all (tci, hp, sb) score blocks:
            # the AV matmuls for block i are emitted after the QK pair of
            # block i+LAG, so by the time the PE reaches them their Exp has
            # finished -- the PE stream never head-of-line blocks, across
            # hp/tci boundaries included.
            LAG = 2
            slots = [(tci, hp, sb) for tci in range(NTC) for hp in range(2)
                     for sb in range(NSB)]
            avs_map = {}
            attn_map = {}
            exs = [None] * len(slots)
            for i in range(len(slots) + LAG):
                if i < len(slots):
                    tci, hp, sb = slots[i]
                    if sb == 0:
                        if hp == 0:
                            attn_map[tci] = [
                                attnp.tile([P, EG], BF16, tag=f"ao{ts}",
                                           name=f"ao_{tci}_{ts}")
                                for ts in range(4)]
                        avs_map[(tci, hp)] = [
                            psav.tile([P, 4 * (D + 1)], F32, tag=f"av{k}",
                                      name=f"av_{tci}_{hp}_{k}")
                            for k in range(2)]
                    exs[i] = emit_qk(tci, hp, sb)
                    if tci == 0 and hp == 0:
                        vproj(sb)
                j = i - LAG
                if j >= 0:
                    tci, hp, sb = slots[j]
                    emit_av(avs_map[(tci, hp)], hp, sb, exs[j])
                    exs[j] = None
                    if sb == NSB - 1:
                        emit_norm(avs_map.pop((tci, hp)), hp, attn_map[tci])
                        if hp == 0 and tci > 0:
                            # previous chunk's out-projection: emitted here so
                            # its psf copy-waits overlap ACT-bound stretches.
                            outproj_tci(tci - 1, attn_map.pop(tci - 1))
            outproj_tci(NTC - 1, attn_map.pop(NTC - 1))

    nc.compile()
    return nc


_NC = None
_LAST_IN_MAPS = None


def _get_nc():
    global _NC
    if _NC is None:
        _NC = _build()
    return _NC


def kernel(**inputs):
    query = np.asarray(inputs["query"], np.float32)
    key_ = np.asarray(inputs["key_"] if "key_" in inputs else inputs["key"],
                      np.float32)
    value = np.asarray(inputs["value"], np.float32)
    Wq = np.asarray(inputs["Wq"], np.float32)
    bq = np.asarray(inputs["bq"], np.float32)
    Wk = np.asarray(inputs["Wk"], np.float32)
    bk = np.asarray(inputs["bk"], np.float32)
    Wv = np.asarray(inputs["Wv"], np.float32)
    bv = np.asarray(inputs["bv"], np.float32)
    Wo = np.asarray(inputs["Wo"], np.float32)
    bo = np.asarray(inputs["bo"], np.float32)
    r = float(np.asarray(inputs["r"]).reshape(-1)[0])

    r_s = 4.0 / (1.0 + np.exp(-np.float64(r)))
    c = np.float32(r_s * r_s / 8.0)

    WqT = Wq.T.astype(BF16NP)
    WkT = Wk.T.astype(BF16NP)
    WoT = Wo.T.astype(BF16NP)
    WvTa = np.concatenate([Wv.T, bv[None, :]], axis=0).astype(BF16NP)

    in_maps = []
    for b in range(B):
        xqT = np.ascontiguousarray(query[b].T).astype(BF16NP)
        xkT = np.ascontiguousarray(key_[b].T).astype(BF16NP)
        xvT = np.concatenate(
            [np.ascontiguousarray(value[b].T), np.ones((1, S), np.float32)],
            axis=0).astype(BF16NP)
        for g in range(HG):
            cols = slice(g * EG, (g + 1) * EG)
            in_maps.append(dict(
                xqT=xqT, xkT=xkT, xvT=xvT,
                wqT=np.ascontiguousarray(WqT[:, cols]),
                wkT=np.ascontiguousarray(WkT[:, cols]),
                wvT=np.ascontiguousarray(WvTa[:, cols]),
                woT=np.ascontiguousarray(WoT[cols, :]),
                bq=np.ascontiguousarray(0.5 * bq[cols, None]),
                bk=np.ascontiguousarray(0.5 * bk[cols, None]),
                cexp=np.array([[c]], np.float32),
            ))

    global _LAST_IN_MAPS
    _LAST_IN_MAPS = in_maps
    res = run_bass_kernel_spmd(_get_nc(), in_maps, core_ids=list(range(NCORES)))
    out = np.empty((B, T, E), np.float32)
    for b in range(B):
        out[b] = res.results[HG * b]["out"]
        for g in range(1, HG):
            out[b] += res.results[HG * b + g]["out"]
        out[b] += bo[None, :]
    return out


# revision 18
# speedup vs baseline: 1.1675x; 1.1675x over previous
"""Trainium2 Bass kernel for ChaoticAttentionLayer.

Math (reference):
    q = r_s * sig(zq) * (1 - sig(zq)),  zq = query @ Wq.T + bq,  r_s = 4*sigmoid(r)
    k likewise, v = value @ Wv.T + bv
    out = softmax(q k^T / 8) v @ Wo.T + bo   (per head, D=64)

Device decomposition:
    g = sig*(1-sig); scores = (r_s^2/8) * g(zq) . g(zk); the r_s^2/8 factor is
    folded into the Exp activation's scale. Scores are bounded in [0, 8] for
    any r, so softmax runs max-free: exp(scores) directly, denominator via an
    all-ones column appended to V.

Sharding: 8 cores = 4 batches x 2 head-groups (4 heads each). Each core
computes partial out[b] = attn_hg @ Wo[:, hg].T; host sums the two partials
per batch and adds bo.

v3 structure (vs the original baseline):
  - All projections in a prologue; K/Q projections accumulate a full
    [128, 2048] psum tile (4 banks) so the Tanh activation runs at N=2048
    (352-cycle ACT fixed cost amortized 4x better).
  - Attention emission is software-pipelined: the AV matmuls for score
    block sb are emitted AFTER the QK matmuls of block sb+1, so the PE
    never head-of-line blocks on the Exp of the current block. This keeps
    the PE stream dense, which keeps the HAM clock-gate at K=8/8 (the
    baseline oscillated and ran the PE at 1.2 GHz for ~57% of the kernel).
  - Exp stays at N=1024 per call (PSUM bank budget forbids larger), which
    makes ScalarE the pacing engine at ~147us.
"""

import numpy as np
import ml_dtypes
from contextlib import ExitStack

try:
    import concourse.bass as bass
except ImportError:  # pragma: no cover
    import sys

    sys.path.insert(0, "/opt/trn_rl_repo")
    import concourse.bass as bass

import concourse.bacc as bacc
import concourse.tile as tile
from concourse import mybir
from concourse.bass_utils import run_bass_kernel_spmd
from concourse.masks import make_identity

F32 = mybir.dt.float32
BF16 = mybir.dt.bfloat16
AF = mybir.ActivationFunctionType
BF16NP = ml_dtypes.bfloat16

B, T, S, E, H = 4, 2048, 2048, 512, 8
D = E // H           # 64 head dim
HG = 2               # head-groups per batch (cores per batch)
HPG = H // HG        # 4 heads per group
EG = HPG * D         # 256 dims per head group
NCORES = 8
P = 128              # partitions
TCH = 512            # t-chunk (psum free dim)
NSB = S // P         # 16 s-blocks
NKT = E // P         # 4 contraction tiles of 128
NTC = T // TCH       # 4 t-chunks


def _build():
    nc = bacc.Bacc("TRN2", target_bir_lowering=False, debug=False,
                   num_devices=NCORES)

    xqT = nc.dram_tensor("xqT", [E, T], BF16, kind="ExternalInput")
    xkT = nc.dram_tensor("xkT", [E, S], BF16, kind="ExternalInput")
    xvT = nc.dram_tensor("xvT", [E + 1, S], BF16, kind="ExternalInput")
    wqT = nc.dram_tensor("wqT", [E, EG], BF16, kind="ExternalInput")
    wkT = nc.dram_tensor("wkT", [E, EG], BF16, kind="ExternalInput")
    wvT = nc.dram_tensor("wvT", [E + 1, EG], BF16, kind="ExternalInput")
    woT = nc.dram_tensor("woT", [EG, E], BF16, kind="ExternalInput")
    bq = nc.dram_tensor("bq", [EG, 1], F32, kind="ExternalInput")
    bk = nc.dram_tensor("bk", [EG, 1], F32, kind="ExternalInput")
    cexp = nc.dram_tensor("cexp", [1, 1], F32, kind="ExternalInput")
    out = nc.dram_tensor("out", [T, E], F32, kind="ExternalOutput")

    with tile.TileContext(nc) as tc, ExitStack() as ctx:
        persist = ctx.enter_context(tc.tile_pool(name="persist", bufs=1))

        # DMA issue engines round-robin: each engine's dma_start lands on its
        # own hardware queue, so input loads run on 5 queues in parallel
        # (a single queue sustains only ~160 GB/s).
        dma_engs = [nc.sync, nc.gpsimd, nc.scalar]

        # --- persistent SBUF state ---
        # x inputs as [128, 1024] half-tiles, round-robined across the three
        # queues in consumption order (all of xk first -- it gates K-proj,
        # which gates everything).
        def load_x(dram, tagbase, nrows=E):
            tiles = []
            idx = 0
            for kt in range(NKT):
                halves = []
                for h in range(2):
                    xt = persist.tile([P, S // 2], BF16,
                                      tag=f"{tagbase}{kt}_{h}",
                                      name=f"{tagbase}{kt}_{h}")
                    dma_engs[idx % 3].dma_start(
                        out=xt,
                        in_=dram[kt * P:(kt + 1) * P,
                                 h * (S // 2):(h + 1) * (S // 2)])
                    halves.append(xt)
                    idx += 1
                tiles.append(halves)
            return tiles

        xk_sb = load_x(xkT, "xk")
        wk_sb = []
        bk_sb = []
        for kt in range(NKT):
            tk = persist.tile([P, EG], BF16, tag=f"wk{kt}")
            dma_engs[kt % 3].dma_start(out=tk, in_=wkT[kt * P:(kt + 1) * P, :])
            wk_sb.append(tk)
        for c in range(EG // P):
            tb2 = persist.tile([P, 1], F32, tag=f"bk{c}")
            nc.sync.dma_start(out=tb2, in_=bk[c * P:(c + 1) * P, :])
            bk_sb.append(tb2)

        xq_sb = load_x(xqT, "xq")
        wq_sb = []
        bq_sb = []
        for kt in range(NKT):
            tq = persist.tile([P, EG], BF16, tag=f"wq{kt}")
            dma_engs[kt % 3].dma_start(out=tq, in_=wqT[kt * P:(kt + 1) * P, :])
            wq_sb.append(tq)
        for c in range(EG // P):
            tb_ = persist.tile([P, 1], F32, tag=f"bq{c}")
            nc.sync.dma_start(out=tb_, in_=bq[c * P:(c + 1) * P, :])
            bq_sb.append(tb_)

        xv_sb = load_x(xvT, "xv")
        wv_sb = []
        for kt in range(NKT):
            tv = persist.tile([P, EG], BF16, tag=f"wv{kt}")
            dma_engs[kt % 3].dma_start(out=tv, in_=wvT[kt * P:(kt + 1) * P, :])
            wv_sb.append(tv)
        wv4_sb = persist.tile([1, EG], BF16, tag="wv4")
        nc.sync.dma_start(out=wv4_sb, in_=wvT[E:E + 1, :])

        ones_sb = persist.tile([1, S], BF16, tag="ones")
        nc.sync.dma_start(out=ones_sb, in_=xvT[E:E + 1, :])

        cexp_sb = persist.tile([P, 1], F32, tag="cexp")
        cap = cexp[:, :]
        nc.sync.dma_start(
            out=cexp_sb,
            in_=bass.AP(tensor=cap.tensor, offset=cap.offset, ap=[[0, P], [1, 1]]),
        )

        # out-proj weights are only needed ~100us in; identity likewise.
        wo_sb = []
        for kb in range(EG // P):
            to = persist.tile([P, E], BF16, tag=f"wo{kb}")
            nc.sync.dma_start(out=to, in_=woT[kb * P:(kb + 1) * P, :])
            wo_sb.append(to)

        ident = persist.tile([P, P], BF16, tag="ident")
        make_identity(nc, ident)

        # PE warm-up: the HAM clock gate starts at K=4/8 (1.2 GHz) and only
        # reaches 2.4 GHz after ~3.4us of sustained PE activity. While the
        # input DMAs stream in, run dummy matmuls on the identity tile so the
        # projections start at full clock instead of paying the cold tax.
        NWARM = 128

        # projected tensors, resident for the whole kernel.
        # Full-width [P, 2048] tiles: free dim is s (for K) / t (for Q).
        QT_sb = [persist.tile([P, T], BF16, tag=f"qt{c}", name=f"qt{c}")
                 for c in range(EG // P)]
        KT_sb = [persist.tile([P, S], BF16, tag=f"kt{c}", name=f"ktile{c}")
                 for c in range(EG // P)]
        V_sb = [persist.tile([P, HPG, D + 1], BF16, tag=f"v{sc}", name=f"v{sc}")
                for sc in range(NSB)]

        # --- K/Q projections (prologue) ---
        # Accumulate all 4 t-chunks into one [128, 2048] psum tile (4 banks)
        # so Tanh runs at N=2048. V-projection is interleaved into the first
        # attention t-chunk (it borrows the psf bank, which the out-proj does
        # not need until tci=1).
        def kq_proj(x_sb, w_sb, b_sb, out_tiles, pool, sig):
            for c in range(EG // P):
                ps = pool.tile([P, T], F32, tag="pskq", name=f"pskq_{c}")
                for tcq in range(NTC):
                    for kt in range(NKT):
                        nc.tensor.matmul(
                            ps[:, tcq * TCH:(tcq + 1) * TCH],
                            w_sb[kt][:, c * P:(c + 1) * P],
                            x_sb[kt][tcq // 2][:, (tcq % 2) * TCH:
                                               (tcq % 2 + 1) * TCH],
                            start=(kt == 0), stop=(kt == NKT - 1))
                # sig'(z) = (1 - tanh^2(z/2)) / 4 -- tanh shares ACT's exp
                # table set, so the whole kernel needs one ACT_TABLE_LOAD.
                y = sig.tile([P, T], F32, tag="y", name=f"y_{c}")
                nc.scalar.activation(y, ps, AF.Tanh, bias=b_sb[c], scale=0.5)
                y2 = sig.tile([P, T], F32, tag="y2", name=f"y2_{c}")
                nc.vector.tensor_mul(y2, y, y)
                nc.vector.tensor_scalar(out_tiles[c], y2, -0.25, 0.25,
                                        mybir.AluOpType.mult,
                                        mybir.AluOpType.add)

        with ExitStack() as cp:
            pkq = cp.enter_context(
                tc.tile_pool(name="pkq", bufs=2, space="PSUM"))
            sig = cp.enter_context(tc.tile_pool(name="sig", bufs=2))
            warm = pkq.tile([P, T], F32, tag="pskq", name="warm")
            for _ in range(NWARM):
                nc.tensor.matmul(warm[:, 0:P], ident, ident,
                                 start=True, stop=True)
            kq_proj(xk_sb, wk_sb, bk_sb, KT_sb, pkq, sig)
            kq_proj(xq_sb, wq_sb, bq_sb, QT_sb, pkq, sig)
            # keep the PE active through the Q post-processing (tanh + DVE)
            # tail so it does not re-throttle right before attention.
            warm2 = pkq.tile([P, T], F32, tag="pskq", name="warm2")
            for _ in range(32):
                nc.tensor.matmul(warm2[:, 0:P], ident, ident,
                                 start=True, stop=True)

        # --- attention + out-projection ---
        # Per (tci, hp): 16 score blocks. Emission is software-pipelined:
        # AV for block sb-1 is emitted after the QK pair of block sb, so the
        # PE stream never blocks on the Exp of the block just produced.
        with ExitStack() as c4:
            pss = c4.enter_context(
                tc.tile_pool(name="pss", bufs=2, space="PSUM"))
            psav = c4.enter_context(
                tc.tile_pool(name="psav", bufs=1, space="PSUM"))
            pst = c4.enter_context(
                tc.tile_pool(name="pst", bufs=1, space="PSUM"))
            psf = c4.enter_context(
                tc.tile_pool(name="psf", bufs=1, space="PSUM"))
            expp = c4.enter_context(tc.tile_pool(name="expp", bufs=6))
            attnp = c4.enter_context(tc.tile_pool(name="attnp", bufs=3))
            atp = c4.enter_context(tc.tile_pool(name="atp", bufs=2))
            rdp = c4.enter_context(tc.tile_pool(name="rdp", bufs=8))
            outp = c4.enter_context(tc.tile_pool(name="outp", bufs=3))

            def emit_qk(tci, hp, sb):
                ps = pss.tile([P, 2 * TCH], F32, tag="sc",
                              name=f"ps_{tci}_{hp}_{sb}")
                for hi in range(2):
                    h = 2 * hp + hi
                    ch, off = h // 2, (h % 2) * D
                    nc.tensor.matmul(
                        ps[:, hi * TCH:(hi + 1) * TCH],
                        KT_sb[ch][off:off + D, sb * P:(sb + 1) * P],
                        QT_sb[ch][off:off + D, tci * TCH:(tci + 1) * TCH],
                        start=True, stop=True,
                        tile_position=(off, 0))
                ex = expp.tile([P, 2 * TCH], BF16, tag="ex")
                nc.scalar.activation(ex, ps, AF.Exp, scale=cexp_sb)
                return ex

            def vproj(sc):
                # V-projection for s-block sc; borrows the psf bank, which
                # out-proj does not touch until tci=1.
                ps = psf.tile([P, E], F32, tag="pf", name=f"psv_{sc}")
                for kt in range(NKT):
                    nc.tensor.matmul(ps[:, 0:EG],
                                     xv_sb[kt][sc // 8][:, (sc % 8) * P:
                                                        (sc % 8 + 1) * P],
                                     wv_sb[kt], start=(kt == 0), stop=False)
                nc.tensor.matmul(ps[:, 0:EG], ones_sb[:, sc * P:(sc + 1) * P],
                                 wv4_sb, start=False, stop=True)
                nc.vector.tensor_copy(
                    V_sb[sc][:, :, 0:D],
                    ps[:, 0:EG].rearrange("p (h d) -> p h d", h=HPG))
                nc.vector.memset(V_sb[sc][:, :, D:D + 1], 1.0)

            def emit_av(avs, hp, sb, ex, attn_ts):
                # On the final s-block, the normalization for each (head,
                # t-sub) block is emitted right after that block's closing
                # matmul, so the DVE drains the accumulators incrementally
                # and the next hp's first AV (which reuses the psav banks)
                # barely waits.
                for hi in range(2):
                    for ts in range(4):
                        nc.tensor.matmul(
                            avs[hi][:, ts * (D + 1):(ts + 1) * (D + 1)],
                            ex[:, hi * TCH + ts * P:hi * TCH + (ts + 1) * P],
                            V_sb[sb][:, 2 * hp + hi, :],
                            start=(sb == 0 and ts == 0),
                            stop=(sb == NSB - 1),
                            skip_group_check=not (sb == 0 and ts == 0))
                        if sb == NSB - 1:
                            h = 2 * hp + hi
                            col = ts * (D + 1)
                            rd = rdp.tile([P, 1], F32, tag="rd")
                            nc.vector.reciprocal(
                                rd, avs[hi][:, col + D:col + D + 1])
                            nc.vector.tensor_scalar_mul(
                                attn_ts[ts][:, h * D:(h + 1) * D],
                                avs[hi][:, col:col + D], rd)

            def transpose_thunk(tci, ts, kb, attn_ts, aT):
                def th():
                    pt = pst.tile([P, P], BF16, tag="pt",
                                  name=f"pt_{tci}_{ts}_{kb}")
                    nc.tensor.transpose(
                        pt, attn_ts[ts][:, kb * P:(kb + 1) * P], ident)
                    nc.vector.tensor_copy(aT[kb][:, ts * P:(ts + 1) * P], pt)
                return th

            def outproj_thunk(tci, ts, aT):
                def th():
                    pf = psf.tile([P, E], F32, tag="pf", name=f"pf_{tci}_{ts}")
                    for kb in range(EG // P):
                        nc.tensor.matmul(pf, aT[kb][:, ts * P:(ts + 1) * P],
                                         wo_sb[kb], start=(kb == 0),
                                         stop=(kb == EG // P - 1))
                    ot = outp.tile([P, E], F32, tag="ot")
                    nc.vector.tensor_copy(ot, pf)
                    row = (tci * 4 + ts) * P
                    dma_engs[(tci * 4 + ts) % 3].dma_start(
                        out=out[row:row + P, :], in_=ot)
                return th

            # Flat software pipeline over all (tci, hp, sb) score blocks:
            # the AV matmuls for block i are emitted after the QK pair of
            # block i+LAG, so by the time the PE reaches them their Exp has
            # finished -- the PE stream never head-of-line blocks, across
            # hp/tci boundaries included. Transpose/out-proj work is queued
            # as small thunks drained one per slot, so it rides in the PE's
            # ACT-bound slack instead of bunching into a stall at chunk
            # boundaries.
            LAG = 2
            slots = [(tci, hp, sb) for tci in range(NTC) for hp in range(2)
                     for sb in range(NSB)]
            avs_map = {}
            attn_map = {}
            aT_map = {}
            exs = [None] * len(slots)
            pending = []
            for i in range(len(slots) + LAG):
                if i < len(slots):
                    tci, hp, sb = slots[i]
                    if sb == 0:
                        if hp == 0:
                            attn_map[tci] = [
                                attnp.tile([P, EG], BF16, tag=f"ao{ts}",
                                           name=f"ao_{tci}_{ts}")
                                for ts in range(4)]
                        avs_map[(tci, hp)] = [
                            psav.tile([P, 4 * (D + 1)], F32, tag=f"av{k}",
                                      name=f"av_{tci}_{hp}_{k}")
                            for k in range(2)]
                    exs[i] = emit_qk(tci, hp, sb)
                    if tci == 0 and hp == 0:
                        vproj(sb)
                j = i - LAG
                if j >= 0:
                    tci, hp, sb = slots[j]
                    emit_av(avs_map[(tci, hp)], hp, sb, exs[j], attn_map[tci])
                    exs[j] = None
                    if sb == NSB - 1:
                        avs_map.pop((tci, hp))
                        # heads of hp land in column block kb=hp of attn_ts,
                        # so its transposes can go out right after this hp.
                        if hp == 0:
                            aT_map[tci] = [
                                atp.tile([P, TCH], BF16, tag=f"at{kb}",
                                         name=f"at_{tci}_{kb}")
                                for kb in range(EG // P)]
                        for ts in range(4):
                            pending.append(transpose_thunk(
                                tci, ts, hp, attn_map[tci], aT_map[tci]))
                        if hp == 1:
                            for ts in range(4):
                                pending.append(outproj_thunk(
                                    tci, ts, aT_map[tci]))
                    if pending and (tci, hp) != (0, 0):
                        pending.pop(0)()
            for th in pending:
                th()

    nc.compile()
    return nc


_NC = None
_LAST_IN_MAPS = None


def _get_nc():
    global _NC
    if _NC is None:
        _NC = _build()
    return _NC


def kernel(**inputs):
    query = np.asarray(inputs["query"], np.float32)
    key_ = np.asarray(inputs["key_"] if "key_" in inputs else inputs["key"],
                      np.float32)
    value = np.asarray(inputs["value"], np.float32)
    Wq = np.asarray(inputs["Wq"], np.float32)
    bq = np.asarray(inputs["bq"], np.float32)
    Wk = np.asarray(inputs["Wk"], np.float32)
    bk = np.asarray(inputs["bk"], np.float32)
    Wv = np.asarray(inputs["Wv"], np.float32)
    bv = np.asarray(inputs["bv"], np.float32)
    Wo = np.asarray(inputs["Wo"], np.float32)
    bo = np.asarray(inputs["bo"], np.float32)
    r = float(np.asarray(inputs["r"]).reshape(-1)[0])

    r_s = 4.0 / (1.0 + np.exp(-np.float64(r)))
    c = np.float32(r_s * r_s / 8.0)

    WqT = Wq.T.astype(BF16NP)
    WkT = Wk.T.astype(BF16NP)
    WoT = Wo.T.astype(BF16NP)
    WvTa = np.concatenate([Wv.T, bv[None, :]], axis=0).astype(BF16NP)

    in_maps = []
    for b in range(B):
        xqT = np.ascontiguousarray(query[b].T).astype(BF16NP)
        xkT = np.ascontiguousarray(key_[b].T).astype(BF16NP)
        xvT = np.concatenate(
            [np.ascontiguousarray(value[b].T), np.ones((1, S), np.float32)],
            axis=0).astype(BF16NP)
        for g in range(HG):
            cols = slice(g * EG, (g + 1) * EG)
            in_maps.append(dict(
                xqT=xqT, xkT=xkT, xvT=xvT,
                wqT=np.ascontiguousarray(WqT[:, cols]),
                wkT=np.ascontiguousarray(WkT[:, cols]),
                wvT=np.ascontiguousarray(WvTa[:, cols]),
                woT=np.ascontiguousarray(WoT[cols, :]),
                bq=np.ascontiguousarray(0.5 * bq[cols, None]),
                bk=np.ascontiguousarray(0.5 * bk[cols, None]),
                cexp=np.array([[c]], np.float32),
            ))

    global _LAST_IN_MAPS
    _LAST_IN_MAPS = in_maps
    res = run_bass_kernel_spmd(_get_nc(), in_maps, core_ids=list(range(NCORES)))
    out = np.empty((B, T, E), np.float32)
    for b in range(B):
        out[b] = res.results[HG * b]["out"]
        for g in range(1, HG):
            out[b] += res.results[HG * b + g]["out"]
        out[b] += bo[None, :]
    return out


# revision 22
# speedup vs baseline: 1.2581x; 1.0776x over previous
"""Trainium2 Bass kernel for ChaoticAttentionLayer.

Math (reference):
    q = r_s * sig(zq) * (1 - sig(zq)),  zq = query @ Wq.T + bq,  r_s = 4*sigmoid(r)
    k likewise, v = value @ Wv.T + bv
    out = softmax(q k^T / 8) v @ Wo.T + bo   (per head, D=64)

Device decomposition:
    g = sig*(1-sig); scores = (r_s^2/8) * g(zq) . g(zk); the r_s^2/8 factor is
    folded into the Exp activation's scale. Scores are bounded in [0, 8] for
    any r, so softmax runs max-free: exp(scores) directly, denominator via an
    all-ones column appended to V.

Sharding: 8 cores = 4 batches x 2 head-groups (4 heads each). Each core
computes partial out[b] = attn_hg @ Wo[:, hg].T; host sums the two partials
per batch and adds bo.

v3 structure (vs the original baseline):
  - All projections in a prologue; K/Q projections accumulate a full
    [128, 2048] psum tile (4 banks) so the Tanh activation runs at N=2048
    (352-cycle ACT fixed cost amortized 4x better).
  - Attention emission is software-pipelined: the AV matmuls for score
    block sb are emitted AFTER the QK matmuls of block sb+1, so the PE
    never head-of-line blocks on the Exp of the current block. This keeps
    the PE stream dense, which keeps the HAM clock-gate at K=8/8 (the
    baseline oscillated and ran the PE at 1.2 GHz for ~57% of the kernel).
  - Exp stays at N=1024 per call (PSUM bank budget forbids larger), which
    makes ScalarE the pacing engine at ~147us.
"""

import numpy as np
import ml_dtypes
from contextlib import ExitStack

try:
    import concourse.bass as bass
except ImportError:  # pragma: no cover
    import sys

    sys.path.insert(0, "/opt/trn_rl_repo")
    import concourse.bass as bass

import concourse.bacc as bacc
import concourse.tile as tile
from concourse import mybir
from concourse.bass_utils import run_bass_kernel_spmd
from concourse.masks import make_identity

F32 = mybir.dt.float32
BF16 = mybir.dt.bfloat16
AF = mybir.ActivationFunctionType
BF16NP = ml_dtypes.bfloat16

B, T, S, E, H = 4, 2048, 2048, 512, 8
D = E // H           # 64 head dim
HG = 2               # head-groups per batch (cores per batch)
HPG = H // HG        # 4 heads per group
EG = HPG * D         # 256 dims per head group
NCORES = 8
P = 128              # partitions
TCH = 512            # t-chunk (psum free dim)
NSB = S // P         # 16 s-blocks
NKT = E // P         # 4 contraction tiles of 128
NTC = T // TCH       # 4 t-chunks


def _build():
    nc = bacc.Bacc("TRN2", target_bir_lowering=False, debug=False,
                   num_devices=NCORES)

    xqT = nc.dram_tensor("xqT", [E, T], BF16, kind="ExternalInput")
    xkT = nc.dram_tensor("xkT", [E, S], BF16, kind="ExternalInput")
    xvT = nc.dram_tensor("xvT", [E + 1, S], BF16, kind="ExternalInput")
    wqT = nc.dram_tensor("wqT", [E, EG], BF16, kind="ExternalInput")
    wkT = nc.dram_tensor("wkT", [E, EG], BF16, kind="ExternalInput")
    wvT = nc.dram_tensor("wvT", [E + 1, EG], BF16, kind="ExternalInput")
    woT = nc.dram_tensor("woT", [EG, E], BF16, kind="ExternalInput")
    bq = nc.dram_tensor("bq", [EG, 1], F32, kind="ExternalInput")
    bk = nc.dram_tensor("bk", [EG, 1], F32, kind="ExternalInput")
    cexp = nc.dram_tensor("cexp", [1, 1], F32, kind="ExternalInput")
    out = nc.dram_tensor("out", [T, E], F32, kind="ExternalOutput")

    with tile.TileContext(nc) as tc, ExitStack() as ctx:
        persist = ctx.enter_context(tc.tile_pool(name="persist", bufs=1))

        # DMA issue engines round-robin: each engine's dma_start lands on its
        # own hardware queue, so input loads run on 3 queues in parallel
        # (a single queue sustains only ~160 GB/s).
        dma_engs = [nc.sync, nc.gpsimd, nc.scalar]

        # identity first: it is built on gpsimd (no DMA) and gates the PE
        # warm-up matmuls, so it must precede gpsimd's share of DMA issues.
        ident = persist.tile([P, P], BF16, tag="ident")
        make_identity(nc, ident)

        # --- persistent SBUF state ---
        # x inputs as [128, 1024] half-tiles, round-robined across the three
        # queues in consumption order (all of xk first -- it gates K-proj,
        # which gates everything).
        def load_x(dram, tagbase, nrows=E):
            tiles = []
            idx = 0
            for kt in range(NKT):
                halves = []
                for h in range(2):
                    xt = persist.tile([P, S // 2], BF16,
                                      tag=f"{tagbase}{kt}_{h}",
                                      name=f"{tagbase}{kt}_{h}")
                    dma_engs[idx % 3].dma_start(
                        out=xt,
                        in_=dram[kt * P:(kt + 1) * P,
                                 h * (S // 2):(h + 1) * (S // 2)])
                    halves.append(xt)
                    idx += 1
                tiles.append(halves)
            return tiles

        xk_sb = load_x(xkT, "xk")
        wk_sb = []
        bk_sb = []
        for kt in range(NKT):
            tk = persist.tile([P, EG], BF16, tag=f"wk{kt}")
            dma_engs[kt % 3].dma_start(out=tk, in_=wkT[kt * P:(kt + 1) * P, :])
            wk_sb.append(tk)
        for c in range(EG // P):
            tb2 = persist.tile([P, 1], F32, tag=f"bk{c}")
            nc.sync.dma_start(out=tb2, in_=bk[c * P:(c + 1) * P, :])
            bk_sb.append(tb2)

        xq_sb = load_x(xqT, "xq")
        wq_sb = []
        bq_sb = []
        for kt in range(NKT):
            tq = persist.tile([P, EG], BF16, tag=f"wq{kt}")
            dma_engs[kt % 3].dma_start(out=tq, in_=wqT[kt * P:(kt + 1) * P, :])
            wq_sb.append(tq)
        for c in range(EG // P):
            tb_ = persist.tile([P, 1], F32, tag=f"bq{c}")
            nc.sync.dma_start(out=tb_, in_=bq[c * P:(c + 1) * P, :])
            bq_sb.append(tb_)

        xv_sb = load_x(xvT, "xv")
        wv_sb = []
        for kt in range(NKT):
            tv = persist.tile([P, EG], BF16, tag=f"wv{kt}")
            dma_engs[kt % 3].dma_start(out=tv, in_=wvT[kt * P:(kt + 1) * P, :])
            wv_sb.append(tv)
        wv4_sb = persist.tile([1, EG], BF16, tag="wv4")
        nc.sync.dma_start(out=wv4_sb, in_=wvT[E:E + 1, :])

        ones_sb = persist.tile([1, S], BF16, tag="ones")
        nc.sync.dma_start(out=ones_sb, in_=xvT[E:E + 1, :])

        cexp_sb = persist.tile([P, 1], F32, tag="cexp")
        cap = cexp[:, :]
        nc.sync.dma_start(
            out=cexp_sb,
            in_=bass.AP(tensor=cap.tensor, offset=cap.offset, ap=[[0, P], [1, 1]]),
        )

        # out-proj weights are only needed ~100us in; identity likewise.
        wo_sb = []
        for kb in range(EG // P):
            to = persist.tile([P, E], BF16, tag=f"wo{kb}")
            nc.sync.dma_start(out=to, in_=woT[kb * P:(kb + 1) * P, :])
            wo_sb.append(to)

        # PE warm-up: the HAM clock gate starts at K=4/8 (1.2 GHz) and only
        # reaches 2.4 GHz after ~3.4us of sustained PE activity. While the
        # input DMAs stream in, run dummy matmuls on the identity tile so the
        # projections start at full clock instead of paying the cold tax.
        NWARM = 80

        # projected tensors, resident for the whole kernel.
        # Full-width [P, 2048] tiles: free dim is s (for K) / t (for Q).
        QT_sb = [persist.tile([P, T], BF16, tag=f"qt{c}", name=f"qt{c}")
                 for c in range(EG // P)]
        KT_sb = [persist.tile([P, S], BF16, tag=f"kt{c}", name=f"ktile{c}")
                 for c in range(EG // P)]
        V_sb = [persist.tile([P, HPG, D + 1], BF16, tag=f"v{sc}", name=f"v{sc}")
                for sc in range(NSB)]

        # --- K/Q projections (prologue) ---
        # Accumulate all 4 t-chunks into one [128, 2048] psum tile (4 banks)
        # so Tanh runs at N=2048. V-projection is interleaved into the first
        # attention t-chunk (it borrows the psf bank, which the out-proj does
        # not need until tci=1).
        def kq_proj(x_sb, w_sb, b_sb, out_tiles, pool, sig):
            for c in range(EG // P):
                ps = pool.tile([P, T], F32, tag="pskq", name=f"pskq_{c}")
                for tcq in range(NTC):
                    for kt in range(NKT):
                        nc.tensor.matmul(
                            ps[:, tcq * TCH:(tcq + 1) * TCH],
                            w_sb[kt][:, c * P:(c + 1) * P],
                            x_sb[kt][tcq // 2][:, (tcq % 2) * TCH:
                                               (tcq % 2 + 1) * TCH],
                            start=(kt == 0), stop=(kt == NKT - 1))
                # sig'(z) = (1 - tanh^2(z/2)) / 4 -- tanh shares ACT's exp
                # table set, so the whole kernel needs one ACT_TABLE_LOAD.
                y = sig.tile([P, T], F32, tag="y", name=f"y_{c}")
                nc.scalar.activation(y, ps, AF.Tanh, bias=b_sb[c], scale=0.5)
                y2 = sig.tile([P, T], F32, tag="y2", name=f"y2_{c}")
                nc.vector.tensor_mul(y2, y, y)
                nc.vector.tensor_scalar(out_tiles[c], y2, -0.25, 0.25,
                                        mybir.AluOpType.mult,
                                        mybir.AluOpType.add)

        with ExitStack() as cp:
            pkq = cp.enter_context(
                tc.tile_pool(name="pkq", bufs=2, space="PSUM"))
            sig = cp.enter_context(tc.tile_pool(name="sig", bufs=2))
            warm = pkq.tile([P, T], F32, tag="pskq", name="warm")
            for _ in range(NWARM):
                nc.tensor.matmul(warm[:, 0:P], ident, ident,
                                 start=True, stop=True)
            kq_proj(xk_sb, wk_sb, bk_sb, KT_sb, pkq, sig)
            kq_proj(xq_sb, wq_sb, bq_sb, QT_sb, pkq, sig)
            # keep the PE active through the Q post-processing (tanh + DVE)
            # tail so it does not re-throttle right before attention.
            warm2 = pkq.tile([P, T], F32, tag="pskq", name="warm2")
            for _ in range(32):
                nc.tensor.matmul(warm2[:, 0:P], ident, ident,
                                 start=True, stop=True)

        # --- attention + out-projection ---
        # Per (tci, hp): 16 score blocks. Emission is software-pipelined:
        # AV for block sb-1 is emitted after the QK pair of block sb, so the
        # PE stream never blocks on the Exp of the block just produced.
        with ExitStack() as c4:
            pss = c4.enter_context(
                tc.tile_pool(name="pss", bufs=2, space="PSUM"))
            psav = c4.enter_context(
                tc.tile_pool(name="psav", bufs=1, space="PSUM"))
            pst = c4.enter_context(
                tc.tile_pool(name="pst", bufs=1, space="PSUM"))
            psf = c4.enter_context(
                tc.tile_pool(name="psf", bufs=1, space="PSUM"))
            expp = c4.enter_context(tc.tile_pool(name="expp", bufs=6))
            attnp = c4.enter_context(tc.tile_pool(name="attnp", bufs=3))
            atp = c4.enter_context(tc.tile_pool(name="atp", bufs=2))
            rdp = c4.enter_context(tc.tile_pool(name="rdp", bufs=8))
            avcp = c4.enter_context(tc.tile_pool(name="avcp", bufs=2))
            outp = c4.enter_context(tc.tile_pool(name="outp", bufs=3))

            def emit_qk(tci, hp, sb):
                ps = pss.tile([P, 2 * TCH], F32, tag="sc",
                              name=f"ps_{tci}_{hp}_{sb}")
                for hi in range(2):
                    h = 2 * hp + hi
                    ch, off = h // 2, (h % 2) * D
                    nc.tensor.matmul(
                        ps[:, hi * TCH:(hi + 1) * TCH],
                        KT_sb[ch][off:off + D, sb * P:(sb + 1) * P],
                        QT_sb[ch][off:off + D, tci * TCH:(tci + 1) * TCH],
                        start=True, stop=True,
                        tile_position=(off, 0))
                ex = expp.tile([P, 2 * TCH], BF16, tag="ex")
                nc.scalar.activation(ex, ps, AF.Exp, scale=cexp_sb)
                return ex

            def vproj(sc):
                # V-projection for s-block sc; borrows the psf bank, which
                # out-proj does not touch until tci=1.
                ps = psf.tile([P, E], F32, tag="pf", name=f"psv_{sc}")
                for kt in range(NKT):
                    nc.tensor.matmul(ps[:, 0:EG],
                                     xv_sb[kt][sc // 8][:, (sc % 8) * P:
                                                        (sc % 8 + 1) * P],
                                     wv_sb[kt], start=(kt == 0), stop=False)
                nc.tensor.matmul(ps[:, 0:EG], ones_sb[:, sc * P:(sc + 1) * P],
                                 wv4_sb, start=False, stop=True)
                nc.vector.tensor_copy(
                    V_sb[sc][:, :, 0:D],
                    ps[:, 0:EG].rearrange("p (h d) -> p h d", h=HPG))
                nc.vector.memset(V_sb[sc][:, :, D:D + 1], 1.0)

            def emit_av(avs, hp, sb, ex, attn_ts):
                # On the final s-block, each head's accumulator is copied to
                # SBUF immediately; normalization then reads the copy, so the
                # next hp's first AV (which reuses the psav banks) only waits
                # for the two short copies, not the whole recip/mul chain.
                for hi in range(2):
                    for ts in range(4):
                        nc.tensor.matmul(
                            avs[hi][:, ts * (D + 1):(ts + 1) * (D + 1)],
                            ex[:, hi * TCH + ts * P:hi * TCH + (ts + 1) * P],
                            V_sb[sb][:, 2 * hp + hi, :],
                            start=(sb == 0 and ts == 0),
                            stop=(sb == NSB - 1),
                            skip_group_check=not (sb == 0 and ts == 0))
                    if sb == NSB - 1:
                        avc = avcp.tile([P, 4 * (D + 1)], F32, tag=f"avc{hi}",
                                        name=f"avc_{hp}_{hi}")
                        nc.vector.tensor_copy(avc, avs[hi])
                        h = 2 * hp + hi
                        rd = rdp.tile([P, 4], F32, tag="rd")
                        nc.vector.reciprocal(rd, avc[:, D::(D + 1)])
                        for ts in range(4):
                            nc.vector.tensor_scalar_mul(
                                attn_ts[ts][:, h * D:(h + 1) * D],
                                avc[:, ts * (D + 1):ts * (D + 1) + D],
                                rd[:, ts:ts + 1])

            def transpose_thunk(tci, ts, kb, attn_ts, aT):
                def th():
                    pt = pst.tile([P, P], BF16, tag="pt",
                                  name=f"pt_{tci}_{ts}_{kb}")
                    nc.tensor.transpose(
                        pt, attn_ts[ts][:, kb * P:(kb + 1) * P], ident)
                    nc.vector.tensor_copy(aT[kb][:, ts * P:(ts + 1) * P], pt)
                return th

            def outproj_thunk(tci, ts, aT):
                def th():
                    pf = psf.tile([P, E], F32, tag="pf", name=f"pf_{tci}_{ts}")
                    for kb in range(EG // P):
                        nc.tensor.matmul(pf, aT[kb][:, ts * P:(ts + 1) * P],
                                         wo_sb[kb], start=(kb == 0),
                                         stop=(kb == EG // P - 1))
                    ot = outp.tile([P, E], F32, tag="ot")
                    nc.vector.tensor_copy(ot, pf)
                    row = (tci * 4 + ts) * P
                    dma_engs[(tci * 4 + ts) % 3].dma_start(
                        out=out[row:row + P, :], in_=ot)
                return th

            # Flat software pipeline over all (tci, hp, sb) score blocks:
            # the AV matmuls for block i are emitted after the QK pair of
            # block i+LAG, so by the time the PE reaches them their Exp has
            # finished -- the PE stream never head-of-line blocks, across
            # hp/tci boundaries included. Transpose/out-proj work is queued
            # as small thunks drained one per slot, so it rides in the PE's
            # ACT-bound slack instead of bunching into a stall at chunk
            # boundaries.
            LAG = 2
            slots = [(tci, hp, sb) for tci in range(NTC) for hp in range(2)
                     for sb in range(NSB)]
            avs_map = {}
            attn_map = {}
            aT_map = {}
            exs = [None] * len(slots)
            pending = []
            for i in range(len(slots) + LAG):
                if i < len(slots):
                    tci, hp, sb = slots[i]
                    if sb == 0:
                        if hp == 0:
                            attn_map[tci] = [
                                attnp.tile([P, EG], BF16, tag=f"ao{ts}",
                                           name=f"ao_{tci}_{ts}")
                                for ts in range(4)]
                        avs_map[(tci, hp)] = [
                            psav.tile([P, 4 * (D + 1)], F32, tag=f"av{k}",
                                      name=f"av_{tci}_{hp}_{k}")
                            for k in range(2)]
                    exs[i] = emit_qk(tci, hp, sb)
                    if tci == 0 and hp == 0:
                        vproj(sb)
                j = i - LAG
                if j >= 0:
                    tci, hp, sb = slots[j]
                    emit_av(avs_map[(tci, hp)], hp, sb, exs[j], attn_map[tci])
                    exs[j] = None
                    if sb == NSB - 1:
                        avs_map.pop((tci, hp))
                        # heads of hp land in column block kb=hp of attn_ts,
                        # so its transposes can go out right after this hp.
                        if hp == 0:
                            aT_map[tci] = [
                                atp.tile([P, TCH], BF16, tag=f"at{kb}",
                                         name=f"at_{tci}_{kb}")
                                for kb in range(EG // P)]
                        for ts in range(4):
                            pending.append(transpose_thunk(
                                tci, ts, hp, attn_map[tci], aT_map[tci]))
                        if hp == 1:
                            for ts in range(4):
                                pending.append(outproj_thunk(
                                    tci, ts, aT_map[tci]))
                    if pending and (tci, hp) != (0, 0):
                        pending.pop(0)()
            for th in pending:
                th()

    nc.compile()
    return nc


_NC = None
_LAST_IN_MAPS = None


def _get_nc():
    global _NC
    if _NC is None:
        _NC = _build()
    return _NC


def kernel(**inputs):
    query = np.asarray(inputs["query"], np.float32)
    key_ = np.asarray(inputs["key_"] if "key_" in inputs else inputs["key"],
                      np.float32)
    value = np.asarray(inputs["value"], np.float32)
    Wq = np.asarray(inputs["Wq"], np.float32)
    bq = np.asarray(inputs["bq"], np.float32)
    Wk = np.asarray(inputs["Wk"], np.float32)
    bk = np.asarray(inputs["bk"], np.float32)
    Wv = np.asarray(inputs["Wv"], np.float32)
    bv = np.asarray(inputs["bv"], np.float32)
    Wo = np.asarray(inputs["Wo"], np.float32)
    bo = np.asarray(inputs["bo"], np.float32)
    r = float(np.asarray(inputs["r"]).reshape(-1)[0])

    r_s = 4.0 / (1.0 + np.exp(-np.float64(r)))
    c = np.float32(r_s * r_s / 8.0)

    WqT = Wq.T.astype(BF16NP)
    WkT = Wk.T.astype(BF16NP)
    WoT = Wo.T.astype(BF16NP)
    WvTa = np.concatenate([Wv.T, bv[None, :]], axis=0).astype(BF16NP)

    in_maps = []
    for b in range(B):
        xqT = np.ascontiguousarray(query[b].T).astype(BF16NP)
        xkT = np.ascontiguousarray(key_[b].T).astype(BF16NP)
        xvT = np.concatenate(
            [np.ascontiguousarray(value[b].T), np.ones((1, S), np.float32)],
            axis=0).astype(BF16NP)
        for g in range(HG):
            cols = slice(g * EG, (g + 1) * EG)
            in_maps.append(dict(
                xqT=xqT, xkT=xkT, xvT=xvT,
                wqT=np.ascontiguousarray(WqT[:, cols]),
                wkT=np.ascontiguousarray(WkT[:, cols]),
                wvT=np.ascontiguousarray(WvTa[:, cols]),
                woT=np.ascontiguousarray(WoT[cols, :]),
                bq=np.ascontiguousarray(0.5 * bq[cols, None]),
                bk=np.ascontiguousarray(0.5 * bk[cols, None]),
                cexp=np.array([[c]], np.float32),
            ))

    global _LAST_IN_MAPS
    _LAST_IN_MAPS = in_maps
    res = run_bass_kernel_spmd(_get_nc(), in_maps, core_ids=list(range(NCORES)))
    out = np.empty((B, T, E), np.float32)
    for b in range(B):
        out[b] = res.results[HG * b]["out"]
        for g in range(1, HG):
            out[b] += res.results[HG * b + g]["out"]
        out[b] += bo[None, :]
    return out


# revision 27
# speedup vs baseline: 1.3297x; 1.0569x over previous
"""Trainium2 Bass kernel for ChaoticAttentionLayer.

Math (reference):
    q = r_s * sig(zq) * (1 - sig(zq)),  zq = query @ Wq.T + bq,  r_s = 4*sigmoid(r)
    k likewise, v = value @ Wv.T + bv
    out = softmax(q k^T / 8) v @ Wo.T + bo   (per head, D=64)

Device decomposition:
    g = sig*(1-sig); scores = (r_s^2/8) * g(zq) . g(zk); the r_s^2/8 factor is
    folded into the Exp activation's scale. Scores are bounded in [0, 8] for
    any r, so softmax runs max-free: exp(scores) directly, denominator via an
    all-ones column appended to V.

Sharding: 8 cores = 4 batches x 2 head-groups (4 heads each). Each core
computes partial out[b] = attn_hg @ Wo[:, hg].T; host sums the two partials
per batch and adds bo.

v3 structure (vs the original baseline):
  - All projections in a prologue; K/Q projections accumulate a full
    [128, 2048] psum tile (4 banks) so the Tanh activation runs at N=2048
    (352-cycle ACT fixed cost amortized 4x better).
  - Attention emission is software-pipelined: the AV matmuls for score
    block sb are emitted AFTER the QK matmuls of block sb+1, so the PE
    never head-of-line blocks on the Exp of the current block. This keeps
    the PE stream dense, which keeps the HAM clock-gate at K=8/8 (the
    baseline oscillated and ran the PE at 1.2 GHz for ~57% of the kernel).
  - Exp stays at N=1024 per call (PSUM bank budget forbids larger), which
    makes ScalarE the pacing engine at ~147us.
"""

import numpy as np
import ml_dtypes
from contextlib import ExitStack

try:
    import concourse.bass as bass
except ImportError:  # pragma: no cover
    import sys

    sys.path.insert(0, "/opt/trn_rl_repo")
    import concourse.bass as bass

import concourse.bacc as bacc
import concourse.tile as tile
from concourse import mybir
from concourse.bass_utils import run_bass_kernel_spmd
from concourse.masks import make_identity

F32 = mybir.dt.float32
BF16 = mybir.dt.bfloat16
AF = mybir.ActivationFunctionType
BF16NP = ml_dtypes.bfloat16

B, T, S, E, H = 4, 2048, 2048, 512, 8
D = E // H           # 64 head dim
HG = 2               # head-groups per batch (cores per batch)
HPG = H // HG        # 4 heads per group
EG = HPG * D         # 256 dims per head group
NCORES = 8
P = 128              # partitions
TCH = 512            # t-chunk (psum free dim)
NSB = S // P         # 16 s-blocks
NKT = E // P         # 4 contraction tiles of 128
NTC = T // TCH       # 4 t-chunks


def _build():
    nc = bacc.Bacc("TRN2", target_bir_lowering=False, debug=False,
                   num_devices=NCORES)

    xqT = nc.dram_tensor("xqT", [E, T], BF16, kind="ExternalInput")
    xkT = nc.dram_tensor("xkT", [E, S], BF16, kind="ExternalInput")
    xvT = nc.dram_tensor("xvT", [E + 1, S], BF16, kind="ExternalInput")
    wqT = nc.dram_tensor("wqT", [E, EG], BF16, kind="ExternalInput")
    wkT = nc.dram_tensor("wkT", [E, EG], BF16, kind="ExternalInput")
    wvT = nc.dram_tensor("wvT", [E + 1, EG], BF16, kind="ExternalInput")
    woT = nc.dram_tensor("woT", [EG, E], BF16, kind="ExternalInput")
    bq = nc.dram_tensor("bq", [EG, 1], F32, kind="ExternalInput")
    bk = nc.dram_tensor("bk", [EG, 1], F32, kind="ExternalInput")
    cexp = nc.dram_tensor("cexp", [1, 1], F32, kind="ExternalInput")
    out = nc.dram_tensor("out", [T, E], F32, kind="ExternalOutput")

    with tile.TileContext(nc) as tc, ExitStack() as ctx:
        persist = ctx.enter_context(tc.tile_pool(name="persist", bufs=1))

        # DMA issue engines round-robin: each engine's dma_start lands on its
        # own hardware queue, so input loads run on 3 queues in parallel
        # (a single queue sustains only ~160 GB/s).
        dma_engs = [nc.sync, nc.gpsimd, nc.scalar]

        # identity first: it is built on gpsimd (no DMA) and gates the PE
        # warm-up matmuls, so it must precede gpsimd's share of DMA issues.
        ident = persist.tile([P, P], BF16, tag="ident")
        make_identity(nc, ident)

        # --- persistent SBUF state ---
        # x inputs as [128, 1024] half-tiles, round-robined across the three
        # queues in consumption order (all of xk first -- it gates K-proj,
        # which gates everything).
        def load_x(dram, tagbase, nrows=E):
            tiles = []
            idx = 0
            for kt in range(NKT):
                halves = []
                for h in range(2):
                    xt = persist.tile([P, S // 2], BF16,
                                      tag=f"{tagbase}{kt}_{h}",
                                      name=f"{tagbase}{kt}_{h}")
                    dma_engs[idx % 3].dma_start(
                        out=xt,
                        in_=dram[kt * P:(kt + 1) * P,
                                 h * (S // 2):(h + 1) * (S // 2)])
                    halves.append(xt)
                    idx += 1
                tiles.append(halves)
            return tiles

        xk_sb = load_x(xkT, "xk")
        wk_sb = []
        bk_sb = []
        for kt in range(NKT):
            tk = persist.tile([P, EG], BF16, tag=f"wk{kt}")
            dma_engs[kt % 3].dma_start(out=tk, in_=wkT[kt * P:(kt + 1) * P, :])
            wk_sb.append(tk)
        for c in range(EG // P):
            tb2 = persist.tile([P, 1], F32, tag=f"bk{c}")
            nc.sync.dma_start(out=tb2, in_=bk[c * P:(c + 1) * P, :])
            bk_sb.append(tb2)

        xq_sb = load_x(xqT, "xq")
        wq_sb = []
        bq_sb = []
        for kt in range(NKT):
            tq = persist.tile([P, EG], BF16, tag=f"wq{kt}")
            dma_engs[kt % 3].dma_start(out=tq, in_=wqT[kt * P:(kt + 1) * P, :])
            wq_sb.append(tq)
        for c in range(EG // P):
            tb_ = persist.tile([P, 1], F32, tag=f"bq{c}")
            nc.sync.dma_start(out=tb_, in_=bq[c * P:(c + 1) * P, :])
            bq_sb.append(tb_)

        xv_sb = load_x(xvT, "xv")
        wv_sb = []
        for kt in range(NKT):
            tv = persist.tile([P, EG], BF16, tag=f"wv{kt}")
            dma_engs[kt % 3].dma_start(out=tv, in_=wvT[kt * P:(kt + 1) * P, :])
            wv_sb.append(tv)
        wv4_sb = persist.tile([1, EG], BF16, tag="wv4")
        nc.sync.dma_start(out=wv4_sb, in_=wvT[E:E + 1, :])

        ones_sb = persist.tile([1, S], BF16, tag="ones")
        nc.sync.dma_start(out=ones_sb, in_=xvT[E:E + 1, :])

        cexp_sb = persist.tile([P, 1], F32, tag="cexp")
        cap = cexp[:, :]
        nc.sync.dma_start(
            out=cexp_sb,
            in_=bass.AP(tensor=cap.tensor, offset=cap.offset, ap=[[0, P], [1, 1]]),
        )

        # out-proj weights are only needed ~100us in; identity likewise.
        wo_sb = []
        for kb in range(EG // P):
            to = persist.tile([P, E], BF16, tag=f"wo{kb}")
            nc.sync.dma_start(out=to, in_=woT[kb * P:(kb + 1) * P, :])
            wo_sb.append(to)

        # PE warm-up: the HAM clock gate starts at K=4/8 (1.2 GHz) and only
        # reaches 2.4 GHz after ~3.4us of sustained PE activity. While the
        # input DMAs stream in, run dummy matmuls on the identity tile so the
        # projections start at full clock instead of paying the cold tax.
        NWARM = 80

        # projected tensors, resident for the whole kernel.
        # Full-width [P, 2048] tiles: free dim is s (for K) / t (for Q).
        QT_sb = [persist.tile([P, T], BF16, tag=f"qt{c}", name=f"qt{c}")
                 for c in range(EG // P)]
        KT_sb = [persist.tile([P, S], BF16, tag=f"kt{c}", name=f"ktile{c}")
                 for c in range(EG // P)]
        V_sb = [persist.tile([P, HPG, D + 1], BF16, tag=f"v{sc}", name=f"v{sc}")
                for sc in range(NSB)]

        # --- K/Q projections (prologue) ---
        # Accumulate all 4 t-chunks into one [128, 2048] psum tile (4 banks)
        # so Tanh runs at N=2048. V-projection is interleaved into the first
        # attention t-chunk (it borrows the psf bank, which the out-proj does
        # not need until tci=1).
        def kq_proj(x_sb, w_sb, b_sb, out_tiles, pool, sig):
            for c in range(EG // P):
                ps = pool.tile([P, T], F32, tag="pskq", name=f"pskq_{c}")
                for tcq in range(NTC):
                    for kt in range(NKT):
                        nc.tensor.matmul(
                            ps[:, tcq * TCH:(tcq + 1) * TCH],
                            w_sb[kt][:, c * P:(c + 1) * P],
                            x_sb[kt][tcq // 2][:, (tcq % 2) * TCH:
                                               (tcq % 2 + 1) * TCH],
                            start=(kt == 0), stop=(kt == NKT - 1))
                # sig'(z) = (1 - tanh^2(z/2)) / 4 -- tanh shares ACT's exp
                # table set, so the whole kernel needs one ACT_TABLE_LOAD.
                # bf16 intermediates double DVE throughput; the projected
                # tiles are stored bf16 anyway.
                y = sig.tile([P, T], BF16, tag="y", name=f"y_{c}")
                nc.scalar.activation(y, ps, AF.Tanh, bias=b_sb[c], scale=0.5)
                y2 = sig.tile([P, T], BF16, tag="y2", name=f"y2_{c}")
                nc.vector.tensor_mul(y2, y, y)
                nc.vector.tensor_scalar(out_tiles[c], y2, -0.25, 0.25,
                                        mybir.AluOpType.mult,
                                        mybir.AluOpType.add)

        with ExitStack() as cp:
            pkq = cp.enter_context(
                tc.tile_pool(name="pkq", bufs=2, space="PSUM"))
            sig = cp.enter_context(tc.tile_pool(name="sig", bufs=2))
            warm = pkq.tile([P, T], F32, tag="pskq", name="warm")
            for _ in range(NWARM):
                nc.tensor.matmul(warm[:, 0:P], ident, ident,
                                 start=True, stop=True)
            kq_proj(xk_sb, wk_sb, bk_sb, KT_sb, pkq, sig)
            kq_proj(xq_sb, wq_sb, bq_sb, QT_sb, pkq, sig)
            # keep the PE active through the Q post-processing (tanh + DVE)
            # tail so it does not re-throttle right before attention.
            warm2 = pkq.tile([P, T], F32, tag="pskq", name="warm2")
            for _ in range(56):
                nc.tensor.matmul(warm2[:, 0:P], ident, ident,
                                 start=True, stop=True)

        # --- attention + out-projection ---
        # Per (tci, hp): 16 score blocks. Emission is software-pipelined:
        # AV for block sb-1 is emitted after the QK pair of block sb, so the
        # PE stream never blocks on the Exp of the block just produced.
        with ExitStack() as c4:
            pss = c4.enter_context(
                tc.tile_pool(name="pss", bufs=2, space="PSUM"))
            psav = c4.enter_context(
                tc.tile_pool(name="psav", bufs=1, space="PSUM"))
            pst = c4.enter_context(
                tc.tile_pool(name="pst", bufs=1, space="PSUM"))
            psf = c4.enter_context(
                tc.tile_pool(name="psf", bufs=1, space="PSUM"))
            expp = c4.enter_context(tc.tile_pool(name="expp", bufs=6))
            attnp = c4.enter_context(tc.tile_pool(name="attnp", bufs=3))
            atp = c4.enter_context(tc.tile_pool(name="atp", bufs=2))
            rdp = c4.enter_context(tc.tile_pool(name="rdp", bufs=8))
            avcp = c4.enter_context(tc.tile_pool(name="avcp", bufs=2))
            outp = c4.enter_context(tc.tile_pool(name="outp", bufs=3))

            def emit_qk(tci, hp, sb):
                ps = pss.tile([P, 2 * TCH], F32, tag="sc",
                              name=f"ps_{tci}_{hp}_{sb}")
                for hi in range(2):
                    h = 2 * hp + hi
                    ch, off = h // 2, (h % 2) * D
                    nc.tensor.matmul(
                        ps[:, hi * TCH:(hi + 1) * TCH],
                        KT_sb[ch][off:off + D, sb * P:(sb + 1) * P],
                        QT_sb[ch][off:off + D, tci * TCH:(tci + 1) * TCH],
                        start=True, stop=True,
                        tile_position=(off, 0))
                ex = expp.tile([P, 2 * TCH], BF16, tag="ex")
                nc.scalar.activation(ex, ps, AF.Exp, scale=cexp_sb)
                return ex

            def vproj(sc):
                # V-projection for s-block sc; borrows the psf bank, which
                # out-proj does not touch until tci=1.
                ps = psf.tile([P, E], F32, tag="pf", name=f"psv_{sc}")
                for kt in range(NKT):
                    nc.tensor.matmul(ps[:, 0:EG],
                                     xv_sb[kt][sc // 8][:, (sc % 8) * P:
                                                        (sc % 8 + 1) * P],
                                     wv_sb[kt], start=(kt == 0), stop=False)
                nc.tensor.matmul(ps[:, 0:EG], ones_sb[:, sc * P:(sc + 1) * P],
                                 wv4_sb, start=False, stop=True)
                nc.vector.tensor_copy(
                    V_sb[sc][:, :, 0:D],
                    ps[:, 0:EG].rearrange("p (h d) -> p h d", h=HPG))
                nc.vector.memset(V_sb[sc][:, :, D:D + 1], 1.0)

            def emit_av(avs, hp, sb, ex, attn_ts):
                # On the final s-block, each head's accumulator is copied to
                # SBUF immediately; normalization then reads the copy, so the
                # next hp's first AV (which reuses the psav banks) only waits
                # for the two short copies, not the whole recip/mul chain.
                for hi in range(2):
                    for ts in range(4):
                        nc.tensor.matmul(
                            avs[hi][:, ts * (D + 1):(ts + 1) * (D + 1)],
                            ex[:, hi * TCH + ts * P:hi * TCH + (ts + 1) * P],
                            V_sb[sb][:, 2 * hp + hi, :],
                            start=(sb == 0 and ts == 0),
                            stop=(sb == NSB - 1),
                            skip_group_check=not (sb == 0 and ts == 0))
                    if sb == NSB - 1:
                        avc = avcp.tile([P, 4 * (D + 1)], F32, tag=f"avc{hi}",
                                        name=f"avc_{hp}_{hi}")
                        nc.vector.tensor_copy(avc, avs[hi])
                        h = 2 * hp + hi
                        rd = rdp.tile([P, 4], F32, tag="rd")
                        nc.vector.reciprocal(rd, avc[:, D::(D + 1)])
                        for ts in range(4):
                            nc.vector.tensor_scalar_mul(
                                attn_ts[ts][:, h * D:(h + 1) * D],
                                avc[:, ts * (D + 1):ts * (D + 1) + D],
                                rd[:, ts:ts + 1])

            def transpose_thunk(tci, ts, kb, attn_ts, aT):
                def th():
                    pt = pst.tile([P, P], BF16, tag="pt",
                                  name=f"pt_{tci}_{ts}_{kb}")
                    nc.tensor.transpose(
                        pt, attn_ts[ts][:, kb * P:(kb + 1) * P], ident)
                    nc.vector.tensor_copy(aT[kb][:, ts * P:(ts + 1) * P], pt)
                return th

            def outproj_thunk(tci, ts, aT):
                def th():
                    pf = psf.tile([P, E], F32, tag="pf", name=f"pf_{tci}_{ts}")
                    for kb in range(EG // P):
                        nc.tensor.matmul(pf, aT[kb][:, ts * P:(ts + 1) * P],
                                         wo_sb[kb], start=(kb == 0),
                                         stop=(kb == EG // P - 1))
                    ot = outp.tile([P, E], F32, tag="ot")
                    nc.vector.tensor_copy(ot, pf)
                    row = (tci * 4 + ts) * P
                    dma_engs[(tci * 4 + ts) % 3].dma_start(
                        out=out[row:row + P, :], in_=ot)
                return th

            # Flat software pipeline over all (tci, hp, sb) score blocks:
            # the AV matmuls for block i are emitted after the QK pair of
            # block i+LAG, so by the time the PE reaches them their Exp has
            # finished -- the PE stream never head-of-line blocks, across
            # hp/tci boundaries included. Transpose/out-proj work is queued
            # as small thunks drained one per slot, so it rides in the PE's
            # ACT-bound slack instead of bunching into a stall at chunk
            # boundaries.
            LAG = 2
            slots = [(tci, hp, sb) for tci in range(NTC) for hp in range(2)
                     for sb in range(NSB)]
            avs_map = {}
            attn_map = {}
            aT_map = {}
            exs = [None] * len(slots)
            pending = []
            for i in range(len(slots) + LAG):
                if i < len(slots):
                    tci, hp, sb = slots[i]
                    if sb == 0:
                        if hp == 0:
                            attn_map[tci] = [
                                attnp.tile([P, EG], BF16, tag=f"ao{ts}",
                                           name=f"ao_{tci}_{ts}")
                                for ts in range(4)]
                        avs_map[(tci, hp)] = [
                            psav.tile([P, 4 * (D + 1)], F32, tag=f"av{k}",
                                      name=f"av_{tci}_{hp}_{k}")
                            for k in range(2)]
                    exs[i] = emit_qk(tci, hp, sb)
                    if tci == 0 and hp == 0:
                        vproj(sb)
                j = i - LAG
                if j >= 0:
                    tci, hp, sb = slots[j]
                    emit_av(avs_map[(tci, hp)], hp, sb, exs[j], attn_map[tci])
                    exs[j] = None
                    if sb == NSB - 1:
                        avs_map.pop((tci, hp))
                        # heads of hp land in column block kb=hp of attn_ts,
                        # so its transposes can go out right after this hp.
                        if hp == 0:
                            aT_map[tci] = [
                                atp.tile([P, TCH], BF16, tag=f"at{kb}",
                                         name=f"at_{tci}_{kb}")
                                for kb in range(EG // P)]
                        for ts in range(4):
                            pending.append(transpose_thunk(
                                tci, ts, hp, attn_map[tci], aT_map[tci]))
                        if hp == 1:
                            for ts in range(4):
                                pending.append(outproj_thunk(
                                    tci, ts, aT_map[tci]))
                    if pending and (tci, hp) != (0, 0):
                        pending.pop(0)()
            # keep the clock warm into the epilogue: the final transposes and
            # out-projection otherwise run at 1.2 GHz.
            wtl = psf.tile([P, P], F32, tag="pf", name="wtl",
                           padded_shape=[P, E])
            for _ in range(12):
                nc.tensor.matmul(wtl, ident, ident, start=True, stop=True)
            for th in pending:
                th()

    nc.compile()
    return nc


_NC = None
_LAST_IN_MAPS = None


def _get_nc():
    global _NC
    if _NC is None:
        _NC = _build()
    return _NC


def kernel(**inputs):
    query = np.asarray(inputs["query"], np.float32)
    key_ = np.asarray(inputs["key_"] if "key_" in inputs else inputs["key"],
                      np.float32)
    value = np.asarray(inputs["value"], np.float32)
    Wq = np.asarray(inputs["Wq"], np.float32)
    bq = np.asarray(inputs["bq"], np.float32)
    Wk = np.asarray(inputs["Wk"], np.float32)
    bk = np.asarray(inputs["bk"], np.float32)
    Wv = np.asarray(inputs["Wv"], np.float32)
    bv = np.asarray(inputs["bv"], np.float32)
    Wo = np.asarray(inputs["Wo"], np.float32)
    bo = np.asarray(inputs["bo"], np.float32)
    r = float(np.asarray(inputs["r"]).reshape(-1)[0])

    r_s = 4.0 / (1.0 + np.exp(-np.float64(r)))
    c = np.float32(r_s * r_s / 8.0)

    WqT = Wq.T.astype(BF16NP)
    WkT = Wk.T.astype(BF16NP)
    WoT = Wo.T.astype(BF16NP)
    WvTa = np.concatenate([Wv.T, bv[None, :]], axis=0).astype(BF16NP)

    in_maps = []
    for b in range(B):
        xqT = np.ascontiguousarray(query[b].T).astype(BF16NP)
        xkT = np.ascontiguousarray(key_[b].T).astype(BF16NP)
        xvT = np.concatenate(
            [np.ascontiguousarray(value[b].T), np.ones((1, S), np.float32)],
            axis=0).astype(BF16NP)
        for g in range(HG):
            cols = slice(g * EG, (g + 1) * EG)
            in_maps.append(dict(
                xqT=xqT, xkT=xkT, xvT=xvT,
                wqT=np.ascontiguousarray(WqT[:, cols]),
                wkT=np.ascontiguousarray(WkT[:, cols]),
                wvT=np.ascontiguousarray(WvTa[:, cols]),
                woT=np.ascontiguousarray(WoT[cols, :]),
                bq=np.ascontiguousarray(0.5 * bq[cols, None]),
                bk=np.ascontiguousarray(0.5 * bk[cols, None]),
                cexp=np.array([[c]], np.float32),
            ))

    global _LAST_IN_MAPS
    _LAST_IN_MAPS = in_maps
    res = run_bass_kernel_spmd(_get_nc(), in_maps, core_ids=list(range(NCORES)))
    out = np.empty((B, T, E), np.float32)
    for b in range(B):
        out[b] = res.results[HG * b]["out"]
        for g in range(1, HG):
            out[b] += res.results[HG * b + g]["out"]
        out[b] += bo[None, :]
    return out


# revision 32
# speedup vs baseline: 1.3968x; 1.0505x over previous
"""Trainium2 Bass kernel for ChaoticAttentionLayer.

Math (reference):
    q = r_s * sig(zq) * (1 - sig(zq)),  zq = query @ Wq.T + bq,  r_s = 4*sigmoid(r)
    k likewise, v = value @ Wv.T + bv
    out = softmax(q k^T / 8) v @ Wo.T + bo   (per head, D=64)

Device decomposition:
    g = sig*(1-sig); scores = (r_s^2/8) * g(zq) . g(zk); the r_s^2/8 factor is
    folded into the Exp activation's scale. Scores are bounded in [0, 8] for
    any r, so softmax runs max-free: exp(scores) directly, denominator via an
    all-ones column appended to V.

Sharding: 8 cores = 4 batches x 2 head-groups (4 heads each). Each core
computes partial out[b] = attn_hg @ Wo[:, hg].T; host sums the two partials
per batch and adds bo.

v3 structure (vs the original baseline):
  - All projections in a prologue; K/Q projections accumulate a full
    [128, 2048] psum tile (4 banks) so the Tanh activation runs at N=2048
    (352-cycle ACT fixed cost amortized 4x better).
  - Attention emission is software-pipelined: the AV matmuls for score
    block sb are emitted AFTER the QK matmuls of block sb+1, so the PE
    never head-of-line blocks on the Exp of the current block. This keeps
    the PE stream dense, which keeps the HAM clock-gate at K=8/8 (the
    baseline oscillated and ran the PE at 1.2 GHz for ~57% of the kernel).
  - Exp stays at N=1024 per call (PSUM bank budget forbids larger), which
    makes ScalarE the pacing engine at ~147us.
"""

import numpy as np
import ml_dtypes
from contextlib import ExitStack

try:
    import concourse.bass as bass
except ImportError:  # pragma: no cover
    import sys

    sys.path.insert(0, "/opt/trn_rl_repo")
    import concourse.bass as bass

import concourse.bacc as bacc
import concourse.tile as tile
from concourse import mybir
from concourse.bass_utils import run_bass_kernel_spmd
from concourse.masks import make_identity

F32 = mybir.dt.float32
BF16 = mybir.dt.bfloat16
AF = mybir.ActivationFunctionType
BF16NP = ml_dtypes.bfloat16

B, T, S, E, H = 4, 2048, 2048, 512, 8
D = E // H           # 64 head dim
HG = 2               # head-groups per batch (cores per batch)
HPG = H // HG        # 4 heads per group
EG = HPG * D         # 256 dims per head group
NCORES = 8
P = 128              # partitions
TCH = 512            # t-chunk (psum free dim)
NSB = S // P         # 16 s-blocks
NKT = E // P         # 4 contraction tiles of 128
NTC = T // TCH       # 4 t-chunks


def _build():
    nc = bacc.Bacc("TRN2", target_bir_lowering=False, debug=False,
                   num_devices=NCORES)

    xqT = nc.dram_tensor("xqT", [E, T], BF16, kind="ExternalInput")
    xkT = nc.dram_tensor("xkT", [E, S], BF16, kind="ExternalInput")
    xvT = nc.dram_tensor("xvT", [E + 1, S], BF16, kind="ExternalInput")
    wqT = nc.dram_tensor("wqT", [E, EG], BF16, kind="ExternalInput")
    wkT = nc.dram_tensor("wkT", [E, EG], BF16, kind="ExternalInput")
    wvT = nc.dram_tensor("wvT", [E + 1, EG], BF16, kind="ExternalInput")
    woT = nc.dram_tensor("woT", [EG, E], BF16, kind="ExternalInput")
    bq = nc.dram_tensor("bq", [EG, 1], F32, kind="ExternalInput")
    bk = nc.dram_tensor("bk", [EG, 1], F32, kind="ExternalInput")
    cexp = nc.dram_tensor("cexp", [1, 1], F32, kind="ExternalInput")
    sch = nc.dram_tensor("sch", [2, 1], F32, kind="ExternalInput")
    out = nc.dram_tensor("out", [T, E], F32, kind="ExternalOutput")

    with tile.TileContext(nc) as tc, ExitStack() as ctx:
        persist = ctx.enter_context(tc.tile_pool(name="persist", bufs=1))

        # DMA issue engines round-robin: each engine's dma_start lands on its
        # own hardware queue, so input loads run on 3 queues in parallel
        # (a single queue sustains only ~160 GB/s).
        dma_engs = [nc.sync, nc.gpsimd, nc.scalar]

        # identity first: it is built on gpsimd (no DMA) and gates the PE
        # warm-up matmuls, so it must precede gpsimd's share of DMA issues.
        ident = persist.tile([P, P], BF16, tag="ident")
        make_identity(nc, ident)

        # --- persistent SBUF state ---
        # x inputs as [128, 1024] half-tiles, round-robined across the three
        # queues in consumption order (all of xk first -- it gates K-proj,
        # which gates everything).
        def load_x(dram, tagbase, nrows=E):
            tiles = []
            idx = 0
            for kt in range(NKT):
                halves = []
                for h in range(2):
                    xt = persist.tile([P, S // 2], BF16,
                                      tag=f"{tagbase}{kt}_{h}",
                                      name=f"{tagbase}{kt}_{h}")
                    dma_engs[idx % 3].dma_start(
                        out=xt,
                        in_=dram[kt * P:(kt + 1) * P,
                                 h * (S // 2):(h + 1) * (S // 2)])
                    halves.append(xt)
                    idx += 1
                tiles.append(halves)
            return tiles

        xk_sb = load_x(xkT, "xk")
        wk_sb = []
        bk_sb = []
        for kt in range(NKT):
            tk = persist.tile([P, EG], BF16, tag=f"wk{kt}")
            dma_engs[kt % 3].dma_start(out=tk, in_=wkT[kt * P:(kt + 1) * P, :])
            wk_sb.append(tk)
        for c in range(EG // P):
            tb2 = persist.tile([P, 1], F32, tag=f"bk{c}")
            nc.sync.dma_start(out=tb2, in_=bk[c * P:(c + 1) * P, :])
            bk_sb.append(tb2)

        xq_sb = load_x(xqT, "xq")
        wq_sb = []
        bq_sb = []
        for kt in range(NKT):
            tq = persist.tile([P, EG], BF16, tag=f"wq{kt}")
            dma_engs[kt % 3].dma_start(out=tq, in_=wqT[kt * P:(kt + 1) * P, :])
            wq_sb.append(tq)
        for c in range(EG // P):
            tb_ = persist.tile([P, 1], F32, tag=f"bq{c}")
            nc.sync.dma_start(out=tb_, in_=bq[c * P:(c + 1) * P, :])
            bq_sb.append(tb_)

        xv_sb = load_x(xvT, "xv")
        wv_sb = []
        for kt in range(NKT):
            tv = persist.tile([P, EG], BF16, tag=f"wv{kt}")
            dma_engs[kt % 3].dma_start(out=tv, in_=wvT[kt * P:(kt + 1) * P, :])
            wv_sb.append(tv)
        wv4_sb = persist.tile([1, EG], BF16, tag="wv4")
        nc.sync.dma_start(out=wv4_sb, in_=wvT[E:E + 1, :])

        ones_sb = persist.tile([1, S], BF16, tag="ones")
        nc.sync.dma_start(out=ones_sb, in_=xvT[E:E + 1, :])

        cexp_sb = persist.tile([P, 1], F32, tag="cexp")
        cap = cexp[:, :]
        nc.sync.dma_start(
            out=cexp_sb,
            in_=bass.AP(tensor=cap.tensor, offset=cap.offset, ap=[[0, P], [1, 1]]),
        )
        # Schraudolph-exp affine constants (per-partition broadcast):
        # bf16_bits(e^(c*u)) ~ round(u * c*2^7/ln2 + (127*2^7 - C))
        sch_sb = []
        for si in range(2):
            st = persist.tile([P, 1], F32, tag=f"sch{si}")
            sap = sch[si:si + 1, :]
            nc.sync.dma_start(
                out=st,
                in_=bass.AP(tensor=sap.tensor, offset=sap.offset,
                            ap=[[0, P], [1, 1]]),
            )
            sch_sb.append(st)

        # out-proj weights are only needed ~100us in; identity likewise.
        wo_sb = []
        for kb in range(EG // P):
            to = persist.tile([P, E], BF16, tag=f"wo{kb}")
            nc.sync.dma_start(out=to, in_=woT[kb * P:(kb + 1) * P, :])
            wo_sb.append(to)

        # PE warm-up: the HAM clock gate starts at K=4/8 (1.2 GHz) and only
        # reaches 2.4 GHz after ~3.4us of sustained PE activity. While the
        # input DMAs stream in, run dummy matmuls on the identity tile so the
        # projections start at full clock instead of paying the cold tax.
        NWARM = 80

        # projected tensors, resident for the whole kernel.
        # Full-width [P, 2048] tiles: free dim is s (for K) / t (for Q).
        QT_sb = [persist.tile([P, T], BF16, tag=f"qt{c}", name=f"qt{c}")
                 for c in range(EG // P)]
        KT_sb = [persist.tile([P, S], BF16, tag=f"kt{c}", name=f"ktile{c}")
                 for c in range(EG // P)]
        V_sb = [persist.tile([P, HPG, D + 1], BF16, tag=f"v{sc}", name=f"v{sc}")
                for sc in range(NSB)]

        # --- K/Q projections (prologue) ---
        # Accumulate all 4 t-chunks into one [128, 2048] psum tile (4 banks)
        # so Tanh runs at N=2048. V-projection is interleaved into the first
        # attention t-chunk (it borrows the psf bank, which the out-proj does
        # not need until tci=1).
        def kq_proj(x_sb, w_sb, b_sb, out_tiles, pool, sig):
            for c in range(EG // P):
                ps = pool.tile([P, T], F32, tag="pskq", name=f"pskq_{c}")
                for tcq in range(NTC):
                    for kt in range(NKT):
                        nc.tensor.matmul(
                            ps[:, tcq * TCH:(tcq + 1) * TCH],
                            w_sb[kt][:, c * P:(c + 1) * P],
                            x_sb[kt][tcq // 2][:, (tcq % 2) * TCH:
                                               (tcq % 2 + 1) * TCH],
                            start=(kt == 0), stop=(kt == NKT - 1))
                # sig'(z) = (1 - tanh^2(z/2)) / 4 -- tanh shares ACT's exp
                # table set, so the whole kernel needs one ACT_TABLE_LOAD.
                # bf16 intermediates double DVE throughput; the projected
                # tiles are stored bf16 anyway.
                y = sig.tile([P, T], BF16, tag="y", name=f"y_{c}")
                nc.scalar.activation(y, ps, AF.Tanh, bias=b_sb[c], scale=0.5)
                y2 = sig.tile([P, T], BF16, tag="y2", name=f"y2_{c}")
                nc.vector.tensor_mul(y2, y, y)
                nc.vector.tensor_scalar(out_tiles[c], y2, -0.25, 0.25,
                                        mybir.AluOpType.mult,
                                        mybir.AluOpType.add)

        with ExitStack() as cp:
            pkq = cp.enter_context(
                tc.tile_pool(name="pkq", bufs=2, space="PSUM"))
            sig = cp.enter_context(tc.tile_pool(name="sig", bufs=2))
            warm = pkq.tile([P, T], F32, tag="pskq", name="warm")
            for _ in range(NWARM):
                nc.tensor.matmul(warm[:, 0:P], ident, ident,
                                 start=True, stop=True)
            kq_proj(xk_sb, wk_sb, bk_sb, KT_sb, pkq, sig)
            kq_proj(xq_sb, wq_sb, bq_sb, QT_sb, pkq, sig)
            # keep the PE active through the Q post-processing (tanh + DVE)
            # tail so it does not re-throttle right before attention.
            warm2 = pkq.tile([P, T], F32, tag="pskq", name="warm2")
            for _ in range(56):
                nc.tensor.matmul(warm2[:, 0:P], ident, ident,
                                 start=True, stop=True)

        # --- attention + out-projection ---
        # Per (tci, hp): 16 score blocks. Emission is software-pipelined:
        # AV for block sb-1 is emitted after the QK pair of block sb, so the
        # PE stream never blocks on the Exp of the block just produced.
        with ExitStack() as c4:
            pss = c4.enter_context(
                tc.tile_pool(name="pss", bufs=2, space="PSUM"))
            psav = c4.enter_context(
                tc.tile_pool(name="psav", bufs=1, space="PSUM"))
            pst = c4.enter_context(
                tc.tile_pool(name="pst", bufs=1, space="PSUM"))
            psf = c4.enter_context(
                tc.tile_pool(name="psf", bufs=1, space="PSUM"))
            expp = c4.enter_context(tc.tile_pool(name="expp", bufs=6))
            attnp = c4.enter_context(tc.tile_pool(name="attnp", bufs=3))
            atp = c4.enter_context(tc.tile_pool(name="atp", bufs=2))
            rdp = c4.enter_context(tc.tile_pool(name="rdp", bufs=8))
            avcp = c4.enter_context(tc.tile_pool(name="avcp", bufs=2))
            outp = c4.enter_context(tc.tile_pool(name="outp", bufs=3))

            # A quarter of the score blocks compute exp on the Vector engine
            # instead of ScalarE (the pacing engine): one tensor_scalar
            # builds the bf16 bit pattern of e^x directly (Schraudolph).
            # Softmax normalization cancels most of the ~1.5% bias.
            SCH_SBS = (2, 6, 10, 14)

            def emit_qk(tci, hp, sb):
                ps = pss.tile([P, 2 * TCH], F32, tag="sc",
                              name=f"ps_{tci}_{hp}_{sb}")
                for hi in range(2):
                    h = 2 * hp + hi
                    ch, off = h // 2, (h % 2) * D
                    nc.tensor.matmul(
                        ps[:, hi * TCH:(hi + 1) * TCH],
                        KT_sb[ch][off:off + D, sb * P:(sb + 1) * P],
                        QT_sb[ch][off:off + D, tci * TCH:(tci + 1) * TCH],
                        start=True, stop=True,
                        tile_position=(off, 0))
                if sb in SCH_SBS:
                    exi = expp.tile([P, 2 * TCH], mybir.dt.int16, tag="exi")
                    nc.vector.tensor_scalar(exi, ps, sch_sb[0], sch_sb[1],
                                            mybir.AluOpType.mult,
                                            mybir.AluOpType.add)
                    return exi.bitcast(BF16)
                ex = expp.tile([P, 2 * TCH], BF16, tag="ex")
                nc.scalar.activation(ex, ps, AF.Exp, scale=cexp_sb)
                return ex

            def vproj(sc):
                # V-projection for s-block sc; borrows the psf bank, which
                # out-proj does not touch until tci=1.
                ps = psf.tile([P, E], F32, tag="pf", name=f"psv_{sc}")
                for kt in range(NKT):
                    nc.tensor.matmul(ps[:, 0:EG],
                                     xv_sb[kt][sc // 8][:, (sc % 8) * P:
                                                        (sc % 8 + 1) * P],
                                     wv_sb[kt], start=(kt == 0), stop=False)
                nc.tensor.matmul(ps[:, 0:EG], ones_sb[:, sc * P:(sc + 1) * P],
                                 wv4_sb, start=False, stop=True)
                nc.vector.tensor_copy(
                    V_sb[sc][:, :, 0:D],
                    ps[:, 0:EG].rearrange("p (h d) -> p h d", h=HPG))
                nc.vector.memset(V_sb[sc][:, :, D:D + 1], 1.0)

            def emit_av(avs, hp, sb, ex, attn_ts):
                # On the final s-block, each head's accumulator is copied to
                # SBUF immediately; normalization then reads the copy, so the
                # next hp's first AV (which reuses the psav banks) only waits
                # for the two short copies, not the whole recip/mul chain.
                for hi in range(2):
                    for ts in range(4):
                        nc.tensor.matmul(
                            avs[hi][:, ts * (D + 1):(ts + 1) * (D + 1)],
                            ex[:, hi * TCH + ts * P:hi * TCH + (ts + 1) * P],
                            V_sb[sb][:, 2 * hp + hi, :],
                            start=(sb == 0 and ts == 0),
                            stop=(sb == NSB - 1),
                            skip_group_check=not (sb == 0 and ts == 0))
                    if sb == NSB - 1:
                        avc = avcp.tile([P, 4 * (D + 1)], F32, tag=f"avc{hi}",
                                        name=f"avc_{hp}_{hi}")
                        nc.vector.tensor_copy(avc, avs[hi])
                        h = 2 * hp + hi
                        rd = rdp.tile([P, 4], F32, tag="rd")
                        nc.vector.reciprocal(rd, avc[:, D::(D + 1)])
                        for ts in range(4):
                            nc.vector.tensor_scalar_mul(
                                attn_ts[ts][:, h * D:(h + 1) * D],
                                avc[:, ts * (D + 1):ts * (D + 1) + D],
                                rd[:, ts:ts + 1])

            def transpose_thunk(tci, ts, kb, attn_ts, aT):
                def th():
                    pt = pst.tile([P, P], BF16, tag="pt",
                                  name=f"pt_{tci}_{ts}_{kb}")
                    nc.tensor.transpose(
                        pt, attn_ts[ts][:, kb * P:(kb + 1) * P], ident)
                    nc.vector.tensor_copy(aT[kb][:, ts * P:(ts + 1) * P], pt)
                return th

            def outproj_thunk(tci, ts, aT):
                def th():
                    pf = psf.tile([P, E], F32, tag="pf", name=f"pf_{tci}_{ts}")
                    for kb in range(EG // P):
                        nc.tensor.matmul(pf, aT[kb][:, ts * P:(ts + 1) * P],
                                         wo_sb[kb], start=(kb == 0),
                                         stop=(kb == EG // P - 1))
                    ot = outp.tile([P, E], F32, tag="ot")
                    nc.vector.tensor_copy(ot, pf)
                    row = (tci * 4 + ts) * P
                    dma_engs[(tci * 4 + ts) % 3].dma_start(
                        out=out[row:row + P, :], in_=ot)
                return th

            # Flat software pipeline over all (tci, hp, sb) score blocks:
            # the AV matmuls for block i are emitted after the QK pair of
            # block i+LAG, so by the time the PE reaches them their Exp has
            # finished -- the PE stream never head-of-line blocks, across
            # hp/tci boundaries included. Transpose/out-proj work is queued
            # as small thunks drained one per slot, so it rides in the PE's
            # ACT-bound slack instead of bunching into a stall at chunk
            # boundaries.
            LAG = 2
            slots = [(tci, hp, sb) for tci in range(NTC) for hp in range(2)
                     for sb in range(NSB)]
            avs_map = {}
            attn_map = {}
            aT_map = {}
            exs = [None] * len(slots)
            pending = []
            for i in range(len(slots) + LAG):
                if i < len(slots):
                    tci, hp, sb = slots[i]
                    if sb == 0:
                        if hp == 0:
                            attn_map[tci] = [
                                attnp.tile([P, EG], BF16, tag=f"ao{ts}",
                                           name=f"ao_{tci}_{ts}")
                                for ts in range(4)]
                        avs_map[(tci, hp)] = [
                            psav.tile([P, 4 * (D + 1)], F32, tag=f"av{k}",
                                      name=f"av_{tci}_{hp}_{k}")
                            for k in range(2)]
                    exs[i] = emit_qk(tci, hp, sb)
                    if tci == 0 and hp == 0:
                        vproj(sb)
                j = i - LAG
                if j >= 0:
                    tci, hp, sb = slots[j]
                    emit_av(avs_map[(tci, hp)], hp, sb, exs[j], attn_map[tci])
                    exs[j] = None
                    if sb == NSB - 1:
                        avs_map.pop((tci, hp))
                        # heads of hp land in column block kb=hp of attn_ts,
                        # so its transposes can go out right after this hp.
                        if hp == 0:
                            aT_map[tci] = [
                                atp.tile([P, TCH], BF16, tag=f"at{kb}",
                                         name=f"at_{tci}_{kb}")
                                for kb in range(EG // P)]
                        for ts in range(4):
                            pending.append(transpose_thunk(
                                tci, ts, hp, attn_map[tci], aT_map[tci]))
                        if hp == 1:
                            for ts in range(4):
                                pending.append(outproj_thunk(
                                    tci, ts, aT_map[tci]))
                    if pending and (tci, hp) != (0, 0):
                        pending.pop(0)()
            # keep the clock warm into the epilogue: the final transposes and
            # out-projection otherwise run at 1.2 GHz.
            wtl = psf.tile([P, P], F32, tag="pf", name="wtl",
                           padded_shape=[P, E])
            for _ in range(12):
                nc.tensor.matmul(wtl, ident, ident, start=True, stop=True)
            for th in pending:
                th()

    nc.compile()
    return nc


_NC = None
_LAST_IN_MAPS = None


def _get_nc():
    global _NC
    if _NC is None:
        _NC = _build()
    return _NC


def kernel(**inputs):
    query = np.asarray(inputs["query"], np.float32)
    key_ = np.asarray(inputs["key_"] if "key_" in inputs else inputs["key"],
                      np.float32)
    value = np.asarray(inputs["value"], np.float32)
    Wq = np.asarray(inputs["Wq"], np.float32)
    bq = np.asarray(inputs["bq"], np.float32)
    Wk = np.asarray(inputs["Wk"], np.float32)
    bk = np.asarray(inputs["bk"], np.float32)
    Wv = np.asarray(inputs["Wv"], np.float32)
    bv = np.asarray(inputs["bv"], np.float32)
    Wo = np.asarray(inputs["Wo"], np.float32)
    bo = np.asarray(inputs["bo"], np.float32)
    r = float(np.asarray(inputs["r"]).reshape(-1)[0])

    r_s = 4.0 / (1.0 + np.exp(-np.float64(r)))
    c = np.float32(r_s * r_s / 8.0)
    # Schraudolph-exp affine constants (bf16 bit pattern construction)
    sch_a1 = np.float64(c) * (2.0 ** 7) / np.log(2.0)
    sch_a0 = 127.0 * 2.0 ** 7 - 5.5
    sch_arr = np.array([[sch_a1], [sch_a0]], np.float32)

    WqT = Wq.T.astype(BF16NP)
    WkT = Wk.T.astype(BF16NP)
    WoT = Wo.T.astype(BF16NP)
    WvTa = np.concatenate([Wv.T, bv[None, :]], axis=0).astype(BF16NP)

    in_maps = []
    for b in range(B):
        xqT = np.ascontiguousarray(query[b].T).astype(BF16NP)
        xkT = np.ascontiguousarray(key_[b].T).astype(BF16NP)
        xvT = np.concatenate(
            [np.ascontiguousarray(value[b].T), np.ones((1, S), np.float32)],
            axis=0).astype(BF16NP)
        for g in range(HG):
            cols = slice(g * EG, (g + 1) * EG)
            in_maps.append(dict(
                xqT=xqT, xkT=xkT, xvT=xvT,
                wqT=np.ascontiguousarray(WqT[:, cols]),
                wkT=np.ascontiguousarray(WkT[:, cols]),
                wvT=np.ascontiguousarray(WvTa[:, cols]),
                woT=np.ascontiguousarray(WoT[cols, :]),
                bq=np.ascontiguousarray(0.5 * bq[cols, None]),
                bk=np.ascontiguousarray(0.5 * bk[cols, None]),
                cexp=np.array([[c]], np.float32),
                sch=sch_arr,
            ))

    global _LAST_IN_MAPS
    _LAST_IN_MAPS = in_maps
    res = run_bass_kernel_spmd(_get_nc(), in_maps, core_ids=list(range(NCORES)))
    out = np.empty((B, T, E), np.float32)
    for b in range(B):
        out[b] = res.results[HG * b]["out"]
        for g in range(1, HG):
            out[b] += res.results[HG * b + g]["out"]
        out[b] += bo[None, :]
    return out
